# revision 1
# baseline (speedup 1.0000x reference)
"""Trainium2 Bass kernel for BottleneckAttention (patch attention).

q patches [160, 5120] from z1_hat (non-overlapping 10x4 unfold),
kv patches [5551, 5120] from z2 (overlapping unfold, Hk=91 x Wk=61),
scores = q @ kv.T / 5120, softmax over kv patches, out = attn @ kv,
folded back to [1, 128, 100, 64].

Sharding: contiguous blocks of 12 kv h-rows per core (8 x 12 = 96 >= 91).
Each core owns the 768 flat positions p = h_local*64 + w (w in [0,64);
positions with w >= 61 or h >= 91 are invalid -- their kv rows are zeroed
so they never touch the numerator, and the host subtracts their exactly
recomputed exp contribution from the denominator. Every core computes all
160 q rows; the host combines with an all-gather softmax.

Per-core kernel (raw Bass, explicit semaphores):
  phase 1 (bf16): scores as implicit convolution against the SBUF-resident
    z2 slice, streamed as CONTIGUOUS 448/320-column flat windows from 4
    byte-shifted copies (one per kernel column offset j), emitted as one
    long PSUM accumulation chain per score tile so the PE pipelines
    back-to-back matmuls. The w>=61 junk columns land on zeroed kv rows
    and are subtracted from the denominator on the host.
  exp on ScalarE (scale = 1/5120), row-sum denominator on VectorE.
  PE transpose of exp-scores; the PSUM->SBUF copy applies bias=-1 so the
  bf16 e_T actually stores f = e-1 (centered softmax: |f| <~ 0.08 keeps
  absolute precision; the host adds the exact sum-of-kv-columns term).
  phase 2 (bf16): partial_out = f_T.T @ kv_shard, kv resident in SBUF.
"""

import sys

sys.path.insert(0, "/opt/trn_rl_repo")

import numpy as np
import ml_dtypes

import concourse.bass as bass
import concourse.mybir as mybir

DT = mybir.dt
AF = mybir.ActivationFunctionType

# problem geometry (hardcoded from the reference module)
KC, KH, KW = 128, 10, 4
H, W = 100, 64
NH, NW = H // KH, W // KW          # 10, 16
PQ = NH * NW                       # 160 q patches
D = KC * KH * KW                   # 5120
HK, WK = H - KH + 1, W - KW + 1    # 91, 61
NCORES = 8
HPC = 12                           # kv h-rows per core (8*12 = 96 >= 91)
PKC = HPC * W                      # 768 flat positions per core
T = 6                              # 768 / 128 k-chunks for phase 2
G0H, G1H = 7, 5                    # phase-1 h-groups (7+5 = 12)
N0 = G0H * W                       # 448: contiguous stream for h 0..6
N1 = G1H * W                       # 320: contiguous stream for h 7..11
OFF1 = G0H * W                     # 448: flat offset of group 1
ZROWS = 2 * HPC                    # 24 z rows staged per core
NT = D // 512                      # 10 phase-2 n-tiles
SCALE = 1.0 / D

P1_NP = ml_dtypes.bfloat16

_CACHE = {}


def _build_nc():
    nc = bass.Bass()
    z_d = nc.declare_dram_parameter("z", [KC, ZROWS * W], DT.bfloat16, isOutput=False)
    q_d = nc.declare_dram_parameter("qT3", [KC, KH * KW, PQ], DT.bfloat16, isOutput=False)
    kv_d = nc.declare_dram_parameter("kvr", [128, T, D], DT.bfloat16, isOutput=False)
    out_d = nc.declare_dram_parameter("out", [PQ, D], DT.float32, isOutput=True)
    den_d = nc.declare_dram_parameter("den", [PQ + 32, 1], DT.float32, isOutput=True)

    from contextlib import ExitStack

    ctx = ExitStack()
    with ctx:
        # 4 byte-shifted copies of flat z so every (i,j) stream is 128B-aligned
        z_sb = ctx.enter_context(nc.sbuf_tensor([KC, KW, ZROWS * W], DT.bfloat16))
        q_sb = ctx.enter_context(nc.sbuf_tensor([KC, KH * KW, PQ], DT.bfloat16))
        kv_sb = ctx.enter_context(nc.sbuf_tensor([128, T, D], DT.bfloat16))
        e_hi = ctx.enter_context(nc.sbuf_tensor([128, PKC], DT.float32))
        e_lo = ctx.enter_context(nc.sbuf_tensor([64, PKC], DT.float32))
        eT_sb = ctx.enter_context(nc.sbuf_tensor([128, T, PQ], DT.bfloat16))
        o_hi = ctx.enter_context(nc.sbuf_tensor([128, D], DT.float32))
        o_lo = ctx.enter_context(nc.sbuf_tensor([64, NT // 2, 512], DT.float32))
        iden = ctx.enter_context(nc.sbuf_tensor([128, 128], DT.float32))
        wz = ctx.enter_context(nc.sbuf_tensor([128, 512], DT.bfloat16))
        bias0 = ctx.enter_context(nc.sbuf_tensor([128, 1], DT.float32))
        dh_sb = ctx.enter_context(nc.sbuf_tensor([128, 1], DT.float32))
        dl_sb = ctx.enter_context(nc.sbuf_tensor([64, 1], DT.float32))

        # phase-1 score accumulators: (h-group, q-half)
        ps_s = [
            ctx.enter_context(nc.psum_tensor(f"ps_s{i}", [128, n], DT.float32))
            for i, n in enumerate((N0, N0, N1, 384))
        ]  # order: g0m0, g0m1, g1m0, g1m1
        # transpose staging / phase-2 accumulators (4 distinct banks)
        ps_t = [
            ctx.enter_context(nc.psum_tensor(f"ps_t{i}", [128, 512], DT.float32))
            for i in range(4)
        ]

        s_z = ctx.enter_context(nc.semaphore("s_z"))
        s_qq = [ctx.enter_context(nc.semaphore(f"s_qq{i}")) for i in range(4)]
        s_kv = ctx.enter_context(nc.semaphore("s_kv"))
        s_p = ctx.enter_context(nc.semaphore("s_p"))
        s_a = ctx.enter_context(nc.semaphore("s_a"))
        s_v = ctx.enter_context(nc.semaphore("s_v"))
        s_g = ctx.enter_context(nc.semaphore("s_g"))
        s_zs = ctx.enter_context(nc.semaphore("s_zs"))
        s_o = ctx.enter_context(nc.semaphore("s_o"))

        # transposes: all 6 m0 chunks (run between the m0 and m1 score
        # chains, keeping the PE busy so HAM stays warm), then the 6 m1
        # chunks after the m1 chains.
        # ACT order / s_a values: exp g0m0=1, exp g1m0=2, m0 copies 3..8,
        # exp g0m1=9, exp g1m1=10, m1 copies 11..16, out-copies 17..36.
        tr_m0 = [(t, 0, 2) for t in range(6)]
        # m1 transposes: (e_lo rows, e_lo col range, iden base, psum rows,
        # eT chunk, eT row range, s_a threshold). The paired m1 chains put
        # the g0 scores at partitions 0-31 and g1 at 32-63, so chunk 3
        # (cols 384..512) splits into two pieces.
        # m1 chains split 384/384 so every transpose chunk is a full,
        # partition-0-aligned 128 columns (transpose psum must start at 0)
        TRM1 = [(t, 9 if t < 3 else 10) for t in range(6)]

        with nc.Block() as block:

            @block.gpsimd
            def _(g):
                g.memset(wz[:], 0.0).then_inc(s_g, 1)        # 1: warmup tile
                g.memset(iden[:], 0.0)
                g.affine_select(
                    out=iden[:],
                    in_=iden[:],
                    compare_op=mybir.AluOpType.not_equal,
                    fill=1.0,
                    base=0,
                    pattern=[[-1, 128]],
                    channel_multiplier=1,
                ).then_inc(s_g, 1)                            # 2: identity
                g.memset(bias0[:], 0.0).then_inc(s_g, 1)      # 3: bias

            @block.sync
            def _(sync):
                sync.dma_start(z_sb[:, 0, :], z_d[:]).then_inc(s_z, 16)
                # q in quarters, each with its own semaphore (completion
                # order across DMA queues is not guaranteed)
                for qtr in range(4):
                    sl = slice(10 * qtr, 10 * qtr + 10)
                    sync.dma_start(q_sb[:, sl, :], q_d[:, sl, :]).then_inc(
                        s_qq[qtr], 16
                    )
                for c in range(3):
                    sync.dma_start(
                        kv_sb[:, 2 * c : 2 * c + 2, :], kv_d[:, 2 * c : 2 * c + 2, :]
                    ).then_inc(s_kv, 16)
                sync.wait_ge(s_v, 1)
                sync.dma_start(den_d[0:128, :], dh_sb[:]).then_inc(s_o, 16)
                sync.wait_ge(s_v, 3)
                sync.dma_start(den_d[128:192, :], dl_sb[:]).then_inc(s_o, 16)
                # out halves pipelined behind the ACT psum->sbuf copies
                # (out-copy g bumps s_a to 17+g; m0 tiles are g 0..9)
                sync.wait_ge(s_a, 21)
                sync.dma_start(out_d[0:128, 0:2560], o_hi[:, 0:2560]).then_inc(s_o, 16)
                sync.wait_ge(s_a, 26)
                sync.dma_start(out_d[0:128, 2560:], o_hi[:, 2560:]).then_inc(s_o, 16)
                out_lo4 = out_d[128:160, :].rearrange(
                    "p (a b c) -> p a b c", a=NT // 2, b=2, c=512
                )
                # pair j's copies land at s_a = 27+2j (even cols) / 28+2j
                sync.wait_ge(s_a, 31)
                sync.dma_start(out_lo4[:, 0:3, 0, :], o_lo[0:32, 0:3, :]).then_inc(
                    s_o, 16
                )
                sync.wait_ge(s_a, 32)
                sync.dma_start(out_lo4[:, 0:3, 1, :], o_lo[32:64, 0:3, :]).then_inc(
                    s_o, 16
                )
                sync.wait_ge(s_a, 35)
                sync.dma_start(out_lo4[:, 3:5, 0, :], o_lo[0:32, 3:5, :]).then_inc(
                    s_o, 16
                )
                sync.wait_ge(s_a, 36)
                sync.dma_start(out_lo4[:, 3:5, 1, :], o_lo[32:64, 3:5, :]).then_inc(
                    s_o, 16
                )
                sync.wait_ge(s_o, 128)

            @block.tensor
            def _(pe):
                # HAM warmup on the zeroed bf16 tile while input DMAs land:
                # phase 1 then starts at the warm 2.4 GHz clock.
                pe.wait_ge(s_g, 1)
                for _w in range(9):
                    nc.tensor.matmul(
                        ps_t[0][0:128, 0:512],
                        wz[:, 0:128],
                        wz[:, 0:512],
                        start=(_w == 0),
                        stop=(_w == 8),
                    )
                pe.wait_ge(s_z, 16)
                pe.wait_ge(s_zs, 3)
                pe.wait_ge(s_qq[0], 16)
                # phase 1: scores[pq, pos] += q(:,ij,:).T @ zflat[:, off+pos]
                # contiguous streams; junk cols (w>=61) corrected on host.
                # One long accumulation chain per psum group -- the PE only
                # pipelines back-to-back matmuls within a group.
                def chain(grp, m, first):
                    ps = ps_s[grp * 2 + m]
                    dst = ps[:, :] if m == 0 else ps[0:32, :]
                    msl = slice(0, 128) if m == 0 else slice(128, 160)
                    for ij in range(KH * KW):
                        if first and ij in (10, 20, 30):
                            pe.wait_ge(s_qq[ij // 10], 16)
                        i_, j_ = ij // KW, ij % KW
                        st, sp = ij == 0, ij == KH * KW - 1
                        base = i_ * W + (OFF1 if grp == 1 else 0)
                        rhs = z_sb[:, j_, base : base + (N1 if grp == 1 else N0)]
                        mm = nc.tensor.matmul(
                            dst, q_sb[:, ij, msl], rhs, start=st, stop=sp
                        )
                    return mm

                def transposes(batch, k0, frees):
                    for k, (t, m, thr) in enumerate(batch, start=k0):
                        msz = 128 if m == 0 else 32
                        esrc = (
                            e_hi[:, t * 128 : (t + 1) * 128]
                            if m == 0
                            else e_lo[:, t * 128 : (t + 1) * 128]
                        )
                        # ps_t[k%4] free once ACT copy k-4 ran
                        freed = 0 if k < 4 else frees[k - 4]
                        pe.wait_ge(s_a, max(thr, freed))
                        nc.tensor.matmul(
                            ps_t[k % 4][0:128, 0:msz],
                            esrc,
                            iden[0:msz, 0:msz],
                            is_transpose=True,
                            start=True,
                            stop=True,
                        ).then_inc(s_p, 1)

                # ACT copy k lands at s_a: m0 k0-5 -> 3..8, m1 k6-12 ->
                # 11..17 (exps at 1, 2, 9, 10)
                COPY_SA = [3, 4, 5, 6, 7, 8, 11, 12, 13, 14, 15, 16]
                chain(0, 0, True).then_inc(s_p, 1)   # s_p = 1
                chain(1, 0, False).then_inc(s_p, 1)  # s_p = 2
                pe.wait_ge(s_g, 2)
                transposes(tr_m0, 0, COPY_SA)        # s_p = 3..8
                # m1 score chains: g0 and g1 run concurrently in disjoint
                # 32-wide PE column groups (psum bases 0 and 32)
                for ij in range(KH * KW):
                    i_, j_ = ij // KW, ij % KW
                    st, sp_ = ij == 0, ij == KH * KW - 1
                    mmA = nc.tensor.matmul(
                        ps_s[1][0:32, 0:384],
                        q_sb[:, ij, 128:160],
                        z_sb[:, j_, i_ * W : i_ * W + 384],
                        start=st,
                        stop=sp_,
                    )
                    mmB = nc.tensor.matmul(
                        ps_s[3][32:64, 0:384],
                        q_sb[:, ij, 128:160],
                        z_sb[:, j_, i_ * W + 384 : i_ * W + 768],
                        start=st,
                        stop=sp_,
                    )
                mmA.then_inc(s_p, 1)  # s_p = 9
                mmB.then_inc(s_p, 1)  # s_p = 10
                for k, (t, thr) in enumerate(TRM1, start=6):
                    freed = COPY_SA[k - 4]
                    pe.wait_ge(s_a, max(thr, freed))
                    rsl = slice(0, 32) if t < 3 else slice(32, 64)
                    ib = 0 if t < 3 else 32
                    nc.tensor.matmul(
                        ps_t[k % 4][0:128, 0:32],
                        e_lo[rsl, t * 128 : (t + 1) * 128],
                        iden[ib : ib + 32, ib : ib + 32],
                        is_transpose=True,
                        start=True,
                        stop=True,
                    ).then_inc(s_p, 1)  # s_p = 11..16
                # phase 2: out[pq, d] = sum_t fT[., t, pq].T @ kv[., t, d]
                pe.wait_ge(s_a, 16)
                pe.wait_ge(s_kv, 48)
                for gidx in range(NT):
                    if gidx >= 4:
                        pe.wait_ge(s_a, 13 + gidx)  # out-copy gidx-4 done
                    for t in range(T):
                        mm = nc.tensor.matmul(
                            ps_t[gidx % 4][0:128, 0:512],
                            eT_sb[:, t, 0:128],
                            kv_sb[:, t, gidx * 512 : (gidx + 1) * 512],
                            start=(t == 0),
                            stop=(t == T - 1),
                        )
                    mm.then_inc(s_p, 1)  # s_p = 17+gidx
                # q-rows 128..159 (M=32): pairs of n-tiles run concurrently
                # in disjoint 32-wide PE column groups (tile_position derives
                # from the psum base partition: 0 vs 32)
                for j in range(NT // 2):
                    gA, gB = 10 + 2 * j, 11 + 2 * j
                    pe.wait_ge(s_a, 13 + gA)
                    pe.wait_ge(s_a, 13 + gB)
                    bA, bB = ps_t[gA % 4], ps_t[gB % 4]
                    for t in range(T):
                        mmA = nc.tensor.matmul(
                            bA[0:32, 0:512],
                            eT_sb[:, t, 128:160],
                            kv_sb[:, t, (2 * j) * 512 : (2 * j + 1) * 512],
                            start=(t == 0),
                            stop=(t == T - 1),
                        )
                        mmB = nc.tensor.matmul(
                            bB[32:64, 0:512],
                            eT_sb[:, t, 128:160],
                            kv_sb[:, t, (2 * j + 1) * 512 : (2 * j + 2) * 512],
                            start=(t == 0),
                            stop=(t == T - 1),
                        )
                    mmA.then_inc(s_p, 1)  # s_p = 27+2j
                    mmB.then_inc(s_p, 1)  # s_p = 28+2j

            @block.scalar
            def _(act):
                def expcall(ps, esl, b):
                    nc.scalar.activation(
                        esl, ps, AF.Exp, bias=b, scale=SCALE
                    ).then_inc(s_a, 1)

                def trcopy(batch, k0, poff):
                    # transposed chunks -> f_T = e - 1 (cast to bf16)
                    for k, (t, m, _thr) in enumerate(batch, start=k0):
                        m0, msz = (0, 128) if m == 0 else (128, 32)
                        act.wait_ge(s_p, poff + k)
                        nc.scalar.activation(
                            eT_sb[:, t, m0 : m0 + msz],
                            ps_t[k % 4][0:128, 0:msz],
                            AF.Copy,
                            bias=-1.0,
                        ).then_inc(s_a, 1)

                act.wait_ge(s_g, 3)
                act.wait_ge(s_p, 1)
                expcall(ps_s[0][:, :], e_hi[:, 0:N0], bias0[:, :])        # s_a=1
                act.wait_ge(s_p, 2)
                expcall(ps_s[2][:, :], e_hi[:, OFF1 : OFF1 + N1], bias0[:, :])  # 2
                trcopy(tr_m0, 0, 3)                                      # s_a=3..8
                act.wait_ge(s_p, 9)
                expcall(ps_s[1][0:32, 0:384], e_lo[0:32, 0:384], bias0[0:32, :])  # 9
                act.wait_ge(s_p, 10)
                expcall(
                    ps_s[3][32:64, 0:384],
                    e_lo[32:64, 384:768],
                    bias0[32:64, :],
                )  # s_a=10
                for k, (t, thr) in enumerate(TRM1, start=6):
                    act.wait_ge(s_p, 5 + k)
                    nc.scalar.activation(
                        eT_sb[:, t, 128:160],
                        ps_t[k % 4][0:128, 0:32],
                        AF.Copy,
                        bias=-1.0,
                    ).then_inc(s_a, 1)  # s_a = 11..16
                # copy phase-2 accumulators to out staging
                for gidx in range(NT):
                    act.wait_ge(s_p, 17 + gidx)
                    nc.scalar.activation(
                        o_hi[:, gidx * 512 : (gidx + 1) * 512],
                        ps_t[gidx % 4][0:128, 0:512],
                        AF.Copy,
                    ).then_inc(s_a, 1)  # s_a = 17+gidx
                for j in range(NT // 2):
                    act.wait_ge(s_p, 27 + 2 * j)
                    nc.scalar.activation(
                        o_lo[0:32, j, :], ps_t[(10 + 2 * j) % 4][0:32, 0:512], AF.Copy
                    ).then_inc(s_a, 1)  # s_a = 27+2j
                    act.wait_ge(s_p, 28 + 2 * j)
                    nc.scalar.activation(
                        o_lo[32:64, j, :],
                        ps_t[(11 + 2 * j) % 4][32:64, 0:512],
                        AF.Copy,
                    ).then_inc(s_a, 1)  # s_a = 28+2j

            @block.vector
            def _(dve):
                # build the 3 byte-shifted z slabs on-chip (saves DMA bytes)
                dve.wait_ge(s_z, 16)
                for s in range(1, KW):
                    nc.vector.tensor_copy(
                        z_sb[:, s, 0 : ZROWS * W - s], z_sb[:, 0, s:]
                    ).then_inc(s_zs, 1)
                dve.wait_ge(s_a, 2)
                nc.vector.reduce_sum(
                    dh_sb[:], e_hi[:, :], axis=mybir.AxisListType.X
                ).then_inc(s_v, 1)
                dve.wait_ge(s_a, 9)
                nc.vector.reduce_sum(
                    dl_sb[0:32, :], e_lo[0:32, 0:384], axis=mybir.AxisListType.X
                ).then_inc(s_v, 1)
                dve.wait_ge(s_a, 10)
                nc.vector.reduce_sum(
                    dl_sb[32:64, :],
                    e_lo[32:64, 384:PKC],
                    axis=mybir.AxisListType.X,
                ).then_inc(s_v, 1)

    return nc


def _host_prep(z1_hat, z2):
    z1 = np.asarray(z1_hat, dtype=np.float32)[0]  # [128, 100, 64]
    z2a = np.asarray(z2, dtype=np.float32)[0]

    # q patches [160, 5120] and lhsT layout qT3 [128, 40, 160]
    q = z1.reshape(KC, NH, KH, NW, KW).transpose(1, 3, 0, 2, 4).reshape(PQ, D)
    qT3 = np.ascontiguousarray(
        q.reshape(PQ, KC, KH * KW).transpose(1, 2, 0).astype(P1_NP)
    )

    # padded z2: rows 100..111 zero
    z_pad = np.zeros((KC, 112, W), dtype=np.float32)
    z_pad[:, :H] = z2a

    # sliding kv patches from padded z2
    sw = np.lib.stride_tricks.sliding_window_view(z_pad, (KH, KW), axis=(1, 2))
    # sw: [128, 103, 61, 10, 4]; patch(h, w) = sw[:, h, w]

    q64 = q.astype(np.float64)
    ij_off = (np.arange(KH)[:, None] * W + np.arange(KW)[None, :]).reshape(-1)  # [40]

    in_maps = []
    corrs = []
    for core in range(NCORES):
        h0 = HPC * core
        zf = z_pad[:, h0 : h0 + ZROWS, :].reshape(KC, ZROWS * W)
        # kv rows indexed by flat position p = h_local*64 + w
        kvp = np.zeros((PKC, D), dtype=np.float32)
        hh = np.arange(PKC) // W
        ww = np.arange(PKC) % W
        real = (ww < WK) & (h0 + hh < HK)
        ridx = np.nonzero(real)[0]
        kvp[ridx] = (
            sw[:, h0 + hh[ridx], ww[ridx]].transpose(1, 0, 2, 3).reshape(-1, D)
        )
        kvr = np.ascontiguousarray(
            kvp.reshape(T, 128, D).transpose(1, 0, 2).astype(ml_dtypes.bfloat16)
        )
        in_maps.append(
            {
                "z": np.ascontiguousarray(zf.astype(P1_NP)),
                "qT3": qT3,
                "kvr": kvr,
            }
        )
        # denominator correction: computed-but-invalid columns. The device
        # computes exp(q . window / D) for every position in the two
        # contiguous streams [0,445) and [448,765); positions that are not
        # real patches (w >= 61 or h >= 91) polluted the on-chip row-sum.
        # streams now cover every flat position; invalid = not a real patch
        bad = np.nonzero(~real)[0]
        win = zf.astype(np.float64)[:, bad[:, None] + ij_off[None, :]]  # [128,nb,40]
        patches = win.transpose(1, 0, 2).reshape(len(bad), D)  # d-order (c, i, j)
        s_bad = q64 @ patches.T  # [160, nb]
        corrs.append(np.exp(s_bad * SCALE).sum(axis=1))

    corr = np.sum(corrs, axis=0)
    # centered softmax: device returns f @ kv with f = e - 1; host adds the
    # exact colsum term sum_k kv[k, :] over all real patches (all cores).
    swr = sw[:, :HK, :WK]
    colsum = swr.astype(np.float64).sum(axis=(1, 2)).reshape(D)  # [5120]
    return in_maps, corr, colsum


def kernel(z1_hat, z2):
    from concourse.bass_utils import run_bass_kernel_spmd

    in_maps, corr, colsum = _host_prep(z1_hat, z2)
    if "nc" not in _CACHE:
        _CACHE["nc"] = _build_nc()
    nc = _CACHE["nc"]
    res = run_bass_kernel_spmd(nc, in_maps, list(range(NCORES)))
    num = np.broadcast_to(colsum, (PQ, D)).astype(np.float64).copy()
    den = -corr
    for r in res.results:
        num += r["out"].astype(np.float64)
        dv = r["den"].astype(np.float64)[:, 0]
        den = den + np.concatenate([dv[0:128], dv[128:160] + dv[160:192]])
    out = (num / den[:, None]).astype(np.float32)
    # fold patches back: [160, 5120] -> [1, 128, 100, 64]
    out = out.reshape(NH, NW, KC, KH, KW).transpose(2, 0, 3, 1, 4)
    return np.ascontiguousarray(out.reshape(1, KC, H, W))



# revision 11
# speedup vs baseline: 1.2097x; 1.2097x over previous
"""Trainium2 Bass kernel for BottleneckAttention (patch attention), fp8 edition.

q patches [160, 5120] from z1_hat (non-overlapping 10x4 unfold),
kv patches [5551, 5120] from z2 (overlapping unfold, Hk=91 x Wk=61),
scores = q @ kv.T / 5120, softmax over kv patches, out = attn @ kv,
folded back to [1, 128, 100, 64].

Sharding: contiguous blocks of 12 kv h-rows per core (8 x 12 = 96 >= 91).
Each core owns the 768 flat positions p = h_local*64 + w (w in [0,64));
positions with w >= 61 or h >= 91 are invalid -- their kv rows are zeroed
so they never touch the numerator, and the host subtracts their exactly
recomputed exp contribution from the denominator. Every core computes all
160 q rows; the host combines with an all-gather softmax.

Per-core kernel (raw Bass, explicit semaphores), fp8e4m3 everywhere on
the PE with DoubleRow (K=256) perf mode for the M=128 matmul blocks:
  phase 1: scores as implicit convolution against the SBUF-resident
    z2 slab (zz holds the slab plus a 64-shifted copy so (i, i+1) kernel
    row pairs form clean [128, 2, N] DoubleRow moving operands).
    q rows 0..127 run M=128 DoubleRow; rows 128..159 run as three
    concurrent 32-wide PE column groups (no DoubleRow -- col tiling and
    DoubleRow are mutually exclusive).
  exp on ScalarE with scale=1/5120 and bias=ln(64): e64 = 64*exp(s).
  row-sum denominator (64x) on VectorE; host divides by 64.
  PE transposes of e64 chunks; the ACT psum->sbuf copy applies bias=-64
  so the fp8 fT stores f64 = 64*(e-1) (centered softmax, scaled into
  fp8e4m3's normal range; the host adds the exact sum-of-kv-columns term
  and divides by 64).
  phase 2: partial_out = f64T.T @ kv_shard in fp8 DoubleRow (m0) plus
    three-column-group fp8 (m1), kv resident in SBUF, drained to bf16.
"""

import sys

sys.path.insert(0, "/opt/trn_rl_repo")

import numpy as np
import ml_dtypes

import concourse.bass as bass
import concourse.mybir as mybir

DT = mybir.dt
AF = mybir.ActivationFunctionType
PM = mybir.MatmulPerfMode

# problem geometry (hardcoded from the reference module)
KC, KH, KW = 128, 10, 4
H, W = 100, 64
NH, NW = H // KH, W // KW          # 10, 16
PQ = NH * NW                       # 160 q patches
D = KC * KH * KW                   # 5120
HK, WK = H - KH + 1, W - KW + 1    # 91, 61
NCORES = 8
HPC = 12                           # kv h-rows per core (8*12 = 96 >= 91)
PKC = HPC * W                      # 768 flat positions per core
ZROWS = 2 * HPC                    # 24 z rows staged per core
ZLEN = ZROWS * W                   # 1536
SCALE = 1.0 / D
LN64 = float(np.log(64.0))
F8 = ml_dtypes.float8_e4m3fn

_CACHE = {}


def _build_nc():
    nc = bass.Bass()
    zz_d = nc.declare_dram_parameter("zz", [KC, 2, ZLEN], DT.float8e4, isOutput=False)
    q_d = nc.declare_dram_parameter("q8", [KC, KW, KH, PQ], DT.float8e4, isOutput=False)
    kv_d = nc.declare_dram_parameter("kv8", [128, 6, D], DT.float8e4, isOutput=False)
    wz_d = nc.declare_dram_parameter("wz", [128, 512], DT.float8e4, isOutput=False)
    id_d = nc.declare_dram_parameter("iden", [128, 128], DT.float32, isOutput=False)
    cln_d = nc.declare_dram_parameter("cln", [128, 1], DT.float32, isOutput=False)
    ohi_d = nc.declare_dram_parameter("ohi", [128, D], DT.bfloat16, isOutput=True)
    olo_d = nc.declare_dram_parameter("olo", [96, 4, 512], DT.bfloat16, isOutput=True)
    den_d = nc.declare_dram_parameter("den", [224, 1], DT.float32, isOutput=True)

    from contextlib import ExitStack

    ctx = ExitStack()
    with ctx:
        zz_sb = ctx.enter_context(nc.sbuf_tensor([KC, 2, ZLEN], DT.float8e4))
        q_sb = ctx.enter_context(nc.sbuf_tensor([KC, KW, KH, PQ], DT.float8e4))
        kv_sb = ctx.enter_context(nc.sbuf_tensor([128, 6, D], DT.float8e4))
        wz = ctx.enter_context(nc.sbuf_tensor([128, 512], DT.float8e4))
        iden = ctx.enter_context(nc.sbuf_tensor([128, 128], DT.float32))
        cln = ctx.enter_context(nc.sbuf_tensor([128, 1], DT.float32))
        e_hi = ctx.enter_context(nc.sbuf_tensor([128, PKC], DT.float32))
        e_lo = ctx.enter_context(nc.sbuf_tensor([96, 256], DT.float32))
        fT = ctx.enter_context(nc.sbuf_tensor([128, 6, PQ], DT.float8e4))
        o_hi = ctx.enter_context(nc.sbuf_tensor([128, D], DT.bfloat16))
        o_lo = ctx.enter_context(nc.sbuf_tensor([96, 4, 512], DT.bfloat16))
        dh_sb = ctx.enter_context(nc.sbuf_tensor([128, 1], DT.float32))
        dl_sb = ctx.enter_context(nc.sbuf_tensor([96, 1], DT.float32))
        scr = ctx.enter_context(nc.sbuf_tensor([128, 8], DT.float32))

        # 7 psum banks: 3 score/aux + 4 rotation
        ps_a = ctx.enter_context(nc.psum_tensor("ps_a", [128, 512], DT.float32))
        ps_b = ctx.enter_context(nc.psum_tensor("ps_b", [128, 512], DT.float32))
        ps_m = ctx.enter_context(nc.psum_tensor("ps_m", [128, 512], DT.float32))
        ps_t = [
            ctx.enter_context(nc.psum_tensor(f"ps_t{i}", [128, 512], DT.float32))
            for i in range(4)
        ]

        s_w = ctx.enter_context(nc.semaphore("s_w"))
        s_i = ctx.enter_context(nc.semaphore("s_i"))
        s_z = ctx.enter_context(nc.semaphore("s_z"))
        s_q = ctx.enter_context(nc.semaphore("s_q"))
        s_kv = [ctx.enter_context(nc.semaphore(f"s_kv{i}")) for i in range(3)]
        s_p = ctx.enter_context(nc.semaphore("s_p"))
        s_a = ctx.enter_context(nc.semaphore("s_a"))
        s_v = ctx.enter_context(nc.semaphore("s_v"))
        s_o = ctx.enter_context(nc.semaphore("s_o"))

        # phase-1 m1 / transpose bank plans
        # p1 m1 col-groups: gA->ps_m[0:32], gB->ps_t0[32:64], gC->ps_t1[64:96]
        M1B = [ps_m, ps_t[0], ps_t[1]]
        # m0 transposes k=0..5 -> banks t2,t3,t0,t1,t2,t3; ACT copy k at s_a=6+k
        TR0_BANK = [2, 3, 0, 1, 2, 3]
        TR0_WAIT = [1, 1, 4, 5, 6, 7]  # s_a thresholds (exp avail + bank drain)
        # m1 transposes k=6..11 alternate ps_b / ps_m; copy k at s_a=6+k
        TR1_BANK = [ps_b, ps_m, ps_b, ps_m, ps_b, ps_m]
        TR1_WAIT = [3, 3, 12, 13, 14, 15]
        # phase-2 m0 pairs: pair k covers n-tiles (2k, 2k+1) on banks below
        P2_BANKS = [(2, 3), (0, 1), (2, 3), (0, 1), (2, 3)]
        P2_SA = [11, 9, 11, 9, 11]          # fT-copy/bank-drain thresholds
        P2_SV = [0, 0, 4, 6, 8]             # DVE drain thresholds for bank reuse
        # phase-2 m1 triples on (ps_a, ps_b, ps_m): r3 waits r2's gA drain
        R_SA = [17, 19, 21, 22]
        R_SV = [0, 13, 14, 15]

        with nc.Block() as block:

            @block.sync
            def _(sync):
                sync.dma_start(wz[:, :], wz_d[:]).then_inc(s_w, 16)
                sync.dma_start(iden[:, :], id_d[:]).then_inc(s_i, 16)
                sync.dma_start(cln[:, :], cln_d[:]).then_inc(s_i, 16)
                sync.dma_start(zz_sb[:, :, :], zz_d[:]).then_inc(s_z, 16)
                sync.dma_start(q_sb[:, :, :, :], q_d[:]).then_inc(s_q, 16)
                sync.dma_start(kv_sb[:, :, 0:1536], kv_d[:, :, 0:1536]).then_inc(
                    s_kv[0], 16
                )
                sync.dma_start(kv_sb[:, :, 1536:3584], kv_d[:, :, 1536:3584]).then_inc(
                    s_kv[1], 16
                )
                sync.dma_start(kv_sb[:, :, 3584:5120], kv_d[:, :, 3584:5120]).then_inc(
                    s_kv[2], 16
                )
                sync.wait_ge(s_v, 1)
                sync.dma_start(den_d[0:128, :], dh_sb[:]).then_inc(s_o, 16)
                sync.wait_ge(s_v, 2)
                sync.dma_start(den_d[128:224, :], dl_sb[:]).then_inc(s_o, 16)
                sync.wait_ge(s_v, 7)
                sync.dma_start(ohi_d[:, 0:2560], o_hi[:, 0:2560]).then_inc(s_o, 16)
                sync.wait_ge(s_v, 12)
                sync.dma_start(ohi_d[:, 2560:5120], o_hi[:, 2560:5120]).then_inc(
                    s_o, 16
                )
                sync.wait_ge(s_a, 24)
                sync.wait_ge(s_v, 15)
                sync.dma_start(olo_d[:, 0:3, :], o_lo[:, 0:3, :]).then_inc(s_o, 16)
                sync.dma_start(olo_d[0:32, 3, :], o_lo[0:32, 3, :]).then_inc(s_o, 16)
                sync.wait_ge(s_o, 96)

            @block.tensor
            def _(pe):
                # HAM warmup on the zeroed fp8 tile while input DMAs land.
                pe.wait_ge(s_w, 16)
                for w in range(10):
                    nc.tensor.matmul(
                        ps_a[0:128, 0:512],
                        wz[:, 0:128],
                        wz[:, 0:512],
                        start=(w == 0),
                        stop=(w == 9),
                    )
                pe.wait_ge(s_z, 16)
                pe.wait_ge(s_q, 16)
                # phase 1 m0 (q rows 0..127): DoubleRow over (i, i+1) pairs.
                # scoresT chunk layout: chain A = pos 0:512, chain B = 512:768
                for j in range(KW):
                    for ip in range(5):
                        st = j == 0 and ip == 0
                        sp = j == KW - 1 and ip == 4
                        off = (2 * ip) * W + j
                        mmA = nc.tensor.matmul(
                            ps_a[0:128, 0:512],
                            q_sb[:, j, 2 * ip : 2 * ip + 2, 0:128],
                            zz_sb[:, :, off : off + 512],
                            start=st,
                            stop=sp,
                            perf_mode=PM.DoubleRow,
                        )
                        mmB = nc.tensor.matmul(
                            ps_b[0:128, 0:256],
                            q_sb[:, j, 2 * ip : 2 * ip + 2, 0:128],
                            zz_sb[:, :, off + 512 : off + 768],
                            start=st,
                            stop=sp,
                            perf_mode=PM.DoubleRow,
                        )
                mmA.then_inc(s_p, 1)  # s_p = 1
                mmB.then_inc(s_p, 1)  # s_p = 2
                # phase 1 m1 (q rows 128..159): 3 concurrent 32-col groups
                mfin = [None, None, None]
                for i in range(KH):
                    for j in range(KW):
                        st = i == 0 and j == 0
                        sp = i == KH - 1 and j == KW - 1
                        off = i * W + j
                        for g in range(3):
                            mfin[g] = nc.tensor.matmul(
                                M1B[g][32 * g : 32 * g + 32, 0:256],
                                q_sb[:, j, i, 128:160],
                                zz_sb[:, 0, off + 256 * g : off + 256 * g + 256],
                                start=st,
                                stop=sp,
                            )
                for g in range(3):
                    mfin[g].then_inc(s_p, 1)  # s_p = 3, 4, 5
                # transposes m0: e64[q0:128, pos chunk t] -> ps_t bank
                pe.wait_ge(s_i, 32)
                for k in range(6):
                    pe.wait_ge(s_a, TR0_WAIT[k])
                    nc.tensor.matmul(
                        ps_t[TR0_BANK[k]][0:128, 0:128],
                        e_hi[:, k * 128 : (k + 1) * 128],
                        iden[0:128, 0:128],
                        is_transpose=True,
                        start=True,
                        stop=True,
                    ).then_inc(s_p, 1)  # s_p = 6..11
                # phase 2 m0 pair 0, then m1 transposes slip in, then pairs 1..4

                def p2pair(k):
                    if P2_SA[k]:
                        pe.wait_ge(s_a, P2_SA[k])
                    if P2_SV[k]:
                        pe.wait_ge(s_v, P2_SV[k])
                    for pc in [[0], [0, 1], [1], [1, 2], [2]][k]:
                        pe.wait_ge(s_kv[pc], 16)
                    bA = ps_t[P2_BANKS[k][0]]
                    bB = ps_t[P2_BANKS[k][1]]
                    for tp in range(3):
                        st, sp = tp == 0, tp == 2
                        mA = nc.tensor.matmul(
                            bA[0:128, 0:512],
                            fT[:, 2 * tp : 2 * tp + 2, 0:128],
                            kv_sb[:, 2 * tp : 2 * tp + 2, (2 * k) * 512 : (2 * k + 1) * 512],
                            start=st,
                            stop=sp,
                            perf_mode=PM.DoubleRow,
                        )
                        mB = nc.tensor.matmul(
                            bB[0:128, 0:512],
                            fT[:, 2 * tp : 2 * tp + 2, 0:128],
                            kv_sb[:, 2 * tp : 2 * tp + 2, (2 * k + 1) * 512 : (2 * k + 2) * 512],
                            start=st,
                            stop=sp,
                            perf_mode=PM.DoubleRow,
                        )
                    mA.then_inc(s_p, 1)  # pair0: 12; pairs 1-4: 20+2(k-1)
                    mB.then_inc(s_p, 1)  # pair0: 13; pairs 1-4: 21+2(k-1)

                p2pair(0)
                # transposes m1 (banks ps_b / ps_m, freed by the m1 exps)
                for k in range(6):
                    t = k
                    g = t // 2
                    col = (t % 2) * 128
                    pe.wait_ge(s_a, TR1_WAIT[k])
                    nc.tensor.matmul(
                        TR1_BANK[k][0:128, 0:32],
                        e_lo[32 * g : 32 * g + 32, col : col + 128],
                        iden[32 * g : 32 * g + 32, 32 * g : 32 * g + 32],
                        is_transpose=True,
                        start=True,
                        stop=True,
                    ).then_inc(s_p, 1)  # s_p = 14..19
                for k in range(1, 5):
                    p2pair(k)
                # phase 2 m1: 4 triples of n-tiles on (ps_a, ps_b, ps_m)
                RB = [ps_a, ps_b, ps_m]
                for r in range(4):
                    pe.wait_ge(s_a, R_SA[r])
                    if R_SV[r]:
                        pe.wait_ge(s_v, R_SV[r])
                    for pc in [[0], [1], [1, 2], [2]][r]:
                        pe.wait_ge(s_kv[pc], 16)
                    ng = 3 if r < 3 else 1
                    mfin = [None] * ng
                    for t6 in range(6):
                        st, sp = t6 == 0, t6 == 5
                        for g in range(ng):
                            mfin[g] = nc.tensor.matmul(
                                RB[g][32 * g : 32 * g + 32, 0:512],
                                fT[:, t6, 128:160],
                                kv_sb[:, t6, (3 * r + g) * 512 : (3 * r + g + 1) * 512],
                                start=st,
                                stop=sp,
                            )
                    for g in range(ng):
                        mfin[g].then_inc(s_p, 1)  # s_p = 28..37

            @block.scalar
            def _(act):
                # warm the exp table set during the DMA/warmup window
                act.wait_ge(s_w, 16)
                nc.scalar.activation(
                    scr[:, :], wz[:, 0:8], AF.Exp, bias=0.0, scale=1.0
                )
                # e64 = 64 * exp(s * SCALE)
                act.wait_ge(s_i, 32)
                act.wait_ge(s_p, 1)
                nc.scalar.activation(
                    e_hi[:, 0:512], ps_a[0:128, 0:512], AF.Exp,
                    bias=cln[:, 0:1], scale=SCALE,
                ).then_inc(s_a, 1)  # 1
                act.wait_ge(s_p, 2)
                nc.scalar.activation(
                    e_hi[:, 512:768], ps_b[0:128, 0:256], AF.Exp,
                    bias=cln[:, 0:1], scale=SCALE,
                ).then_inc(s_a, 1)  # 2
                for g in range(3):
                    act.wait_ge(s_p, 3 + g)
                    nc.scalar.activation(
                        e_lo[32 * g : 32 * g + 32, 0:256],
                        M1B[g][32 * g : 32 * g + 32, 0:256],
                        AF.Exp,
                        bias=cln[32 * g : 32 * g + 32, 0:1],
                        scale=SCALE,
                    ).then_inc(s_a, 1)  # 3, 4, 5
                # fT copies: f64 = e64T - 64, cast to fp8
                for k in range(6):
                    act.wait_ge(s_p, 6 + k)
                    nc.scalar.activation(
                        fT[:, k, 0:128],
                        ps_t[TR0_BANK[k]][0:128, 0:128],
                        AF.Copy,
                        bias=-64.0,
                    ).then_inc(s_a, 1)  # 6..11
                for k in range(6):
                    act.wait_ge(s_p, 14 + k)
                    nc.scalar.activation(
                        fT[:, k, 128:160],
                        TR1_BANK[k][0:128, 0:32],
                        AF.Copy,
                        bias=-64.0,
                    ).then_inc(s_a, 1)  # 12..17
                # phase-2 m1 drains (gA, gB per triple; DVE takes gC)
                SP_R = [(28, 29), (31, 32), (34, 35), (37,)]
                RB = [ps_a, ps_b, ps_m]
                for r in range(4):
                    for gi, spv in enumerate(SP_R[r]):
                        act.wait_ge(s_p, spv)
                        nc.scalar.activation(
                            o_lo[32 * gi : 32 * gi + 32, r, :],
                            RB[gi][32 * gi : 32 * gi + 32, 0:512],
                            AF.Copy,
                        ).then_inc(s_a, 1)  # 18..24

            @block.vector
            def _(dve):
                dve.wait_ge(s_a, 2)
                nc.vector.reduce_sum(
                    dh_sb[:], e_hi[:, :], axis=mybir.AxisListType.X
                ).then_inc(s_v, 1)  # 1
                dve.wait_ge(s_a, 5)
                nc.vector.reduce_sum(
                    dl_sb[:], e_lo[:, :], axis=mybir.AxisListType.X
                ).then_inc(s_v, 1)  # 2
                # phase-2 m0 drains to bf16 staging
                NT_SP = [12, 13, 20, 21, 22, 23, 24, 25, 26, 27]
                for g in range(10):
                    dve.wait_ge(s_p, NT_SP[g])
                    bank = ps_t[P2_BANKS[g // 2][g % 2]]
                    nc.vector.tensor_copy(
                        o_hi[:, g * 512 : (g + 1) * 512], bank[0:128, 0:512]
                    ).then_inc(s_v, 1)  # 3..12
                # phase-2 m1 gC drains (r = 0..2)
                for r in range(3):
                    dve.wait_ge(s_p, 30 + 3 * r)
                    nc.vector.tensor_copy(
                        o_lo[64:96, r, :], ps_m[64:96, 0:512]
                    ).then_inc(s_v, 1)  # 13..15

    return nc


def _host_prep(z1_hat, z2):
    z1 = np.asarray(z1_hat, dtype=np.float32)[0]  # [128, 100, 64]
    z2a = np.asarray(z2, dtype=np.float32)[0]

    # q patches [160, 5120]; device layout q8 [128, j, i, 160]
    q = z1.reshape(KC, NH, KH, NW, KW).transpose(1, 3, 0, 2, 4).reshape(PQ, D)
    q4 = q.reshape(PQ, KC, KH, KW)
    q8 = np.ascontiguousarray(q4.transpose(1, 3, 2, 0).astype(F8))  # [c, j, i, p]

    # padded z2: rows 100..111 zero
    z_pad = np.zeros((KC, 112, W), dtype=np.float32)
    z_pad[:, :H] = z2a
    z8_pad = z_pad.astype(F8)

    # sliding kv patches from padded z2 (original fp32 values, cast per-row)
    sw = np.lib.stride_tricks.sliding_window_view(z_pad, (KH, KW), axis=(1, 2))

    q64 = q.astype(np.float64)
    ij_off = (np.arange(KH)[:, None] * W + np.arange(KW)[None, :]).reshape(-1)  # [40]

    wz = np.zeros((128, 512), dtype=F8)
    iden = np.eye(128, dtype=np.float32)
    cln = np.full((128, 1), LN64, dtype=np.float32)

    in_maps = []
    corrs = []
    for core in range(NCORES):
        h0 = HPC * core
        slab8 = z8_pad[:, h0 : h0 + ZROWS, :].reshape(KC, ZLEN)
        zz = np.zeros((KC, 2, ZLEN), dtype=F8)
        zz[:, 0, :] = slab8
        zz[:, 1, 0 : ZLEN - W] = slab8[:, W:]
        # kv rows indexed by flat position p = h_local*64 + w
        kvp = np.zeros((PKC, D), dtype=F8)
        hh = np.arange(PKC) // W
        ww = np.arange(PKC) % W
        real = (ww < WK) & (h0 + hh < HK)
        ridx = np.nonzero(real)[0]
        kvp[ridx] = (
            sw[:, h0 + hh[ridx], ww[ridx]].transpose(1, 0, 2, 3).reshape(-1, D)
        ).astype(F8)
        kv8 = np.ascontiguousarray(kvp.reshape(6, 128, D).transpose(1, 0, 2))
        in_maps.append(
            {"zz": zz, "q8": q8, "kv8": kv8, "wz": wz, "iden": iden, "cln": cln}
        )
        # denominator correction: computed-but-invalid stream positions,
        # recomputed exactly (fp64) from the original values.
        bad = np.nonzero(~real)[0]
        zf = z_pad[:, h0 : h0 + ZROWS, :].reshape(KC, ZLEN).astype(np.float64)
        win = zf[:, bad[:, None] + ij_off[None, :]]  # [128, nb, 40]
        patches = win.transpose(1, 0, 2).reshape(len(bad), D)
        s_bad = q64 @ patches.T  # [160, nb]
        corrs.append(np.exp(s_bad * SCALE).sum(axis=1))

    corr = np.sum(corrs, axis=0)
    swr = sw[:, :HK, :WK]
    colsum = swr.astype(np.float64).sum(axis=(1, 2)).reshape(D)  # [5120]
    return in_maps, corr, colsum


def kernel(z1_hat, z2):
    from concourse.bass_utils import run_bass_kernel_spmd

    in_maps, corr, colsum = _host_prep(z1_hat, z2)
    if "nc" not in _CACHE:
        _CACHE["nc"] = _build_nc()
    nc = _CACHE["nc"]
    res = run_bass_kernel_spmd(nc, in_maps, list(range(NCORES)))
    num = np.broadcast_to(colsum, (PQ, D)).astype(np.float64).copy()
    den = -corr
    for r in res.results:
        ohi = r["ohi"].astype(np.float64)  # [128, 5120] = 64 * partial m0
        olo = r["olo"].astype(np.float64).reshape(96, 4, 512)
        part = np.zeros((PQ, D))
        part[0:128] = ohi
        for nt in range(10):
            rr, g = nt // 3, nt % 3
            part[128:160, nt * 512 : (nt + 1) * 512] = olo[32 * g : 32 * g + 32, rr]
        num += part / 64.0
        dv = r["den"].astype(np.float64)[:, 0] / 64.0
        den = den + np.concatenate(
            [dv[0:128], dv[128:160] + dv[160:192] + dv[192:224]]
        )
    out = (num / den[:, None]).astype(np.float32)
    # fold patches back: [160, 5120] -> [1, 128, 100, 64]
    out = out.reshape(NH, NW, KC, KH, KW).transpose(2, 0, 3, 1, 4)
    return np.ascontiguousarray(out.reshape(1, KC, H, W))


# revision 12
# speedup vs baseline: 1.3516x; 1.1173x over previous
"""Trainium2 Bass kernel for BottleneckAttention (patch attention), fp8 edition.

q patches [160, 5120] from z1_hat (non-overlapping 10x4 unfold),
kv patches [5551, 5120] from z2 (overlapping unfold, Hk=91 x Wk=61),
scores = q @ kv.T / 5120, softmax over kv patches, out = attn @ kv,
folded back to [1, 128, 100, 64].

Sharding: contiguous blocks of 12 kv h-rows per core (8 x 12 = 96 >= 91).
Each core owns the 768 flat positions p = h_local*64 + w (w in [0,64));
positions with w >= 61 or h >= 91 are invalid -- their kv rows are zeroed
so they never touch the numerator, and the host subtracts their exactly
recomputed exp contribution from the denominator. Every core computes all
160 q rows; the host combines with an all-gather softmax.

Per-core kernel (raw Bass, explicit semaphores), fp8e4m3 everywhere on
the PE with DoubleRow (K=256) perf mode for the M=128 matmul blocks:
  phase 1: scores as implicit convolution against the SBUF-resident
    z2 slab (zz holds the slab plus a 64-shifted copy so (i, i+1) kernel
    row pairs form clean [128, 2, N] DoubleRow moving operands).
    q rows 0..127 run M=128 DoubleRow; rows 128..159 run as three
    concurrent 32-wide PE column groups (no DoubleRow -- col tiling and
    DoubleRow are mutually exclusive).
  exp on ScalarE with scale=1/5120 and bias=ln(64): e64 = 64*exp(s).
  row-sum denominator (64x) on VectorE; host divides by 64.
  PE transposes of e64 chunks; the ACT psum->sbuf copy applies bias=-64
  so the fp8 fT stores f64 = 64*(e-1) (centered softmax, scaled into
  fp8e4m3's normal range; the host adds the exact sum-of-kv-columns term
  and divides by 64).
  phase 2: partial_out = f64T.T @ kv_shard in fp8 DoubleRow (m0) plus
    three-column-group fp8 (m1), kv resident in SBUF, drained to bf16.
"""

import sys

sys.path.insert(0, "/opt/trn_rl_repo")

import numpy as np
import ml_dtypes

import concourse.bass as bass
import concourse.mybir as mybir

DT = mybir.dt
AF = mybir.ActivationFunctionType
PM = mybir.MatmulPerfMode

# problem geometry (hardcoded from the reference module)
KC, KH, KW = 128, 10, 4
H, W = 100, 64
NH, NW = H // KH, W // KW          # 10, 16
PQ = NH * NW                       # 160 q patches
D = KC * KH * KW                   # 5120
HK, WK = H - KH + 1, W - KW + 1    # 91, 61
NCORES = 8
HPC = 12                           # kv h-rows per core (8*12 = 96 >= 91)
PKC = HPC * W                      # 768 flat positions per core
ZROWS = 2 * HPC                    # 24 z rows staged per core
ZLEN = ZROWS * W                   # 1536
SCALE = 1.0 / D
LN64 = float(np.log(64.0))
F8 = ml_dtypes.float8_e4m3fn

_CACHE = {}


def _build_nc():
    nc = bass.Bass()
    zz_d = nc.declare_dram_parameter("zz", [KC, 2, ZLEN], DT.float8e4, isOutput=False)
    qm1_d = nc.declare_dram_parameter("qm1", [KC, KW, KH, 32], DT.float8e4, isOutput=False)
    idc_d = nc.declare_dram_parameter("idc", [128, 129], DT.float32, isOutput=False)
    qm0_d = nc.declare_dram_parameter("qm0", [KC, KW, KH, 128], DT.float8e4, isOutput=False)
    kv_d = nc.declare_dram_parameter("kv8", [128, 6, D], DT.float8e4, isOutput=False)
    ohi_d = nc.declare_dram_parameter("ohi", [128, D], DT.bfloat16, isOutput=True)
    olo_d = nc.declare_dram_parameter("olo", [96, 4, 512], DT.bfloat16, isOutput=True)
    den_d = nc.declare_dram_parameter("den", [224, 1], DT.float32, isOutput=True)

    from contextlib import ExitStack

    ctx = ExitStack()
    with ctx:
        zz_sb = ctx.enter_context(nc.sbuf_tensor([KC, 2, ZLEN], DT.float8e4))
        qm1_sb = ctx.enter_context(nc.sbuf_tensor([KC, KW, KH, 32], DT.float8e4))
        qm0_sb = ctx.enter_context(nc.sbuf_tensor([KC, KW, KH, 128], DT.float8e4))
        idc = ctx.enter_context(nc.sbuf_tensor([128, 129], DT.float32))
        kv_sb = ctx.enter_context(nc.sbuf_tensor([128, 6, D], DT.float8e4))
        e_hi = ctx.enter_context(nc.sbuf_tensor([128, PKC], DT.float32))
        e_lo = ctx.enter_context(nc.sbuf_tensor([96, 256], DT.float32))
        fT = ctx.enter_context(nc.sbuf_tensor([128, 6, PQ], DT.float8e4))
        o_hi = ctx.enter_context(nc.sbuf_tensor([128, D], DT.bfloat16))
        o_lo = ctx.enter_context(nc.sbuf_tensor([96, 4, 512], DT.bfloat16))
        dh_sb = ctx.enter_context(nc.sbuf_tensor([128, 1], DT.float32))
        dl_sb = ctx.enter_context(nc.sbuf_tensor([96, 1], DT.float32))
        scr = ctx.enter_context(nc.sbuf_tensor([128, 8], DT.float32))

        ps_a = ctx.enter_context(nc.psum_tensor("ps_a", [128, 512], DT.float32))
        ps_b = ctx.enter_context(nc.psum_tensor("ps_b", [128, 512], DT.float32))
        ps_m = ctx.enter_context(nc.psum_tensor("ps_m", [128, 512], DT.float32))
        ps_w = ctx.enter_context(nc.psum_tensor("ps_w", [128, 512], DT.float32))
        ps_t = [
            ctx.enter_context(nc.psum_tensor(f"ps_t{i}", [128, 512], DT.float32))
            for i in range(4)
        ]

        s_z = ctx.enter_context(nc.semaphore("s_z"))
        s_q1 = ctx.enter_context(nc.semaphore("s_q1"))
        s_q0 = ctx.enter_context(nc.semaphore("s_q0"))
        s_i = ctx.enter_context(nc.semaphore("s_i"))
        s_kv = [ctx.enter_context(nc.semaphore(f"s_kv{i}")) for i in range(3)]
        s_p = ctx.enter_context(nc.semaphore("s_p"))
        s_a = ctx.enter_context(nc.semaphore("s_a"))
        s_v = ctx.enter_context(nc.semaphore("s_v"))
        s_o = ctx.enter_context(nc.semaphore("s_o"))

        # p1 m1 col-groups: gA->ps_m[0:32], gB->ps_t0[32:64], gC->ps_t1[64:96]
        M1B = [ps_m, ps_t[0], ps_t[1]]
        # TR1 transposes (m1 e_lo chunks 0..5) alternate banks t2/t3
        TR1_BANK = [ps_t[2], ps_t[3], ps_t[2], ps_t[3], ps_t[2], ps_t[3]]
        TR1_SA = [1, 1, 2, 2, 3, 3]       # e_lo chunk needs its group's exp
        TR1_SV = [0, 0, 2, 3, 4, 5]       # bank drain (DVE fT copy) for reuse
        # TR0 emission order: e_hi chunks [4,5,0,1,2,3] on banks [t0,t1,t2,t3,t0,t1]
        TR0_CHUNK = [4, 5, 0, 1, 2, 3]
        TR0_BANK = [ps_t[0], ps_t[1], ps_t[2], ps_t[3], ps_t[0], ps_t[1]]
        TR0_SA = [4, 4, 5, 5, 5, 5]       # chunks 4,5 <- exp B; 0..3 <- exp A
        TR0_SV = [0, 0, 6, 7, 8, 9]
        # phase-2 m0 pairs (n-tiles 2k, 2k+1) and m1 triples (n-tiles 3r..3r+2)
        PB = [(2, 3), (0, 1), (2, 3), (0, 1), (2, 3)]
        PAIR_SV = [13, 13, 16, 19, 22]
        PAIR_KV = [[0], [0, 1], [1], [1, 2], [2]]
        PAIR_SP = [(18, 19), (23, 24), (28, 29), (33, 34), (36, 37)]
        RB = [ps_a, ps_b, ps_m]
        TRIP_SA = [5, 7, 9, 10]
        TRIP_SV = [0, 17, 20, 0]
        TRIP_KV = [[0], [1], [1, 2], [2]]
        TRIP_SP = [(20, 21, 22), (25, 26, 27), (30, 31, 32), (35,)]

        with nc.Block() as block:

            @block.sync
            def _(sync):
                sync.dma_start(zz_sb[:, :, :], zz_d[:]).then_inc(s_z, 16)
                sync.dma_start(qm1_sb[:, :, :, :], qm1_d[:]).then_inc(s_q1, 16)
                sync.dma_start(idc[:, :], idc_d[:]).then_inc(s_i, 16)
                sync.dma_start(qm0_sb[:, :, :, :], qm0_d[:]).then_inc(s_q0, 16)
                sync.dma_start(kv_sb[:, :, 0:1536], kv_d[:, :, 0:1536]).then_inc(
                    s_kv[0], 16
                )
                sync.dma_start(kv_sb[:, :, 1536:3584], kv_d[:, :, 1536:3584]).then_inc(
                    s_kv[1], 16
                )
                sync.dma_start(kv_sb[:, :, 3584:5120], kv_d[:, :, 3584:5120]).then_inc(
                    s_kv[2], 16
                )
                sync.wait_ge(s_v, 1)
                sync.dma_start(den_d[128:224, :], dl_sb[:]).then_inc(s_o, 16)
                sync.wait_ge(s_v, 14)
                sync.dma_start(den_d[0:128, :], dh_sb[:]).then_inc(s_o, 16)
                sync.wait_ge(s_v, 16)
                sync.dma_start(ohi_d[:, 0:1024], o_hi[:, 0:1024]).then_inc(s_o, 16)
                sync.wait_ge(s_v, 19)
                sync.dma_start(ohi_d[:, 1024:2048], o_hi[:, 1024:2048]).then_inc(
                    s_o, 16
                )
                sync.wait_ge(s_a, 9)
                sync.wait_ge(s_v, 20)
                sync.dma_start(olo_d[:, 0:2, :], o_lo[:, 0:2, :]).then_inc(s_o, 16)
                sync.wait_ge(s_v, 22)
                sync.dma_start(ohi_d[:, 2048:3072], o_hi[:, 2048:3072]).then_inc(
                    s_o, 16
                )
                sync.wait_ge(s_a, 11)
                sync.wait_ge(s_v, 23)
                sync.dma_start(olo_d[:, 2, :], o_lo[:, 2, :]).then_inc(s_o, 16)
                sync.wait_ge(s_a, 12)
                sync.dma_start(olo_d[0:32, 3, :], o_lo[0:32, 3, :]).then_inc(s_o, 16)
                sync.wait_ge(s_v, 25)
                sync.dma_start(ohi_d[:, 3072:4096], o_hi[:, 3072:4096]).then_inc(
                    s_o, 16
                )
                sync.wait_ge(s_v, 27)
                sync.dma_start(ohi_d[:, 4096:5120], o_hi[:, 4096:5120]).then_inc(
                    s_o, 16
                )
                sync.wait_ge(s_o, 160)

            @block.tensor
            def _(pe):
                # HAM warmup on the zz slab while the q DMAs land
                pe.wait_ge(s_z, 16)
                for w in range(3):
                    nc.tensor.matmul(
                        ps_w[0:128, 0:512],
                        zz_sb[:, 0, 0:128],
                        zz_sb[:, 0, 0:512],
                        start=(w == 0),
                        stop=(w == 2),
                    )
                pe.wait_ge(s_q1, 16)
                # phase 1 m1 (q rows 128..159): 3 concurrent 32-col groups
                mfin = [None, None, None]
                for i in range(KH):
                    for j in range(KW):
                        st = i == 0 and j == 0
                        sp = i == KH - 1 and j == KW - 1
                        off = i * W + j
                        for g in range(3):
                            mfin[g] = nc.tensor.matmul(
                                M1B[g][32 * g : 32 * g + 32, 0:256],
                                qm1_sb[:, j, i, :],
                                zz_sb[:, 0, off + 256 * g : off + 256 * g + 256],
                                start=st,
                                stop=sp,
                            )
                for g in range(3):
                    mfin[g].then_inc(s_p, 1)  # s_p = 1, 2, 3
                pe.wait_ge(s_q0, 16)
                # phase 1 m0 chain B (pos 512:768) then chain A (pos 0:512),
                # DoubleRow over (i, i+1) kernel-row pairs
                for j in range(KW):
                    for ip in range(5):
                        st = j == 0 and ip == 0
                        sp = j == KW - 1 and ip == 4
                        off = (2 * ip) * W + j
                        mm = nc.tensor.matmul(
                            ps_b[0:128, 0:256],
                            qm0_sb[:, j, 2 * ip : 2 * ip + 2, :],
                            zz_sb[:, :, off + 512 : off + 768],
                            start=st,
                            stop=sp,
                            perf_mode=PM.DoubleRow,
                        )
                mm.then_inc(s_p, 1)  # s_p = 4
                for j in range(KW):
                    for ip in range(5):
                        st = j == 0 and ip == 0
                        sp = j == KW - 1 and ip == 4
                        off = (2 * ip) * W + j
                        mm = nc.tensor.matmul(
                            ps_a[0:128, 0:512],
                            qm0_sb[:, j, 2 * ip : 2 * ip + 2, :],
                            zz_sb[:, :, off : off + 512],
                            start=st,
                            stop=sp,
                            perf_mode=PM.DoubleRow,
                        )
                mm.then_inc(s_p, 1)  # s_p = 5
                # TR1: transpose e_lo (m1) chunks -> fp8 fT cols 128:160
                pe.wait_ge(s_i, 16)
                for k in range(6):
                    g = k // 2
                    col = (k % 2) * 128
                    pe.wait_ge(s_a, TR1_SA[k])
                    if TR1_SV[k]:
                        pe.wait_ge(s_v, TR1_SV[k])
                    nc.tensor.matmul(
                        TR1_BANK[k][0:128, 0:32],
                        e_lo[32 * g : 32 * g + 32, col : col + 128],
                        idc[32 * g : 32 * g + 32, 32 * g : 32 * g + 32],
                        is_transpose=True,
                        start=True,
                        stop=True,
                    ).then_inc(s_p, 1)  # s_p = 6..11
                # TR0: transpose e_hi (m0) chunks, B-dependent chunks first
                for k in range(6):
                    c = TR0_CHUNK[k]
                    pe.wait_ge(s_a, TR0_SA[k])
                    if TR0_SV[k]:
                        pe.wait_ge(s_v, TR0_SV[k])
                    nc.tensor.matmul(
                        TR0_BANK[k][0:128, 0:128],
                        e_hi[:, c * 128 : (c + 1) * 128],
                        idc[0:128, 0:128],
                        is_transpose=True,
                        start=True,
                        stop=True,
                    ).then_inc(s_p, 1)  # s_p = 12..17

                def p2pair(k):
                    pe.wait_ge(s_v, PAIR_SV[k])
                    for pc in PAIR_KV[k]:
                        pe.wait_ge(s_kv[pc], 16)
                    bA, bB = ps_t[PB[k][0]], ps_t[PB[k][1]]
                    for tp in range(3):
                        st, sp = tp == 0, tp == 2
                        mA = nc.tensor.matmul(
                            bA[0:128, 0:512],
                            fT[:, 2 * tp : 2 * tp + 2, 0:128],
                            kv_sb[
                                :, 2 * tp : 2 * tp + 2, (2 * k) * 512 : (2 * k + 1) * 512
                            ],
                            start=st,
                            stop=sp,
                            perf_mode=PM.DoubleRow,
                        )
                        mB = nc.tensor.matmul(
                            bB[0:128, 0:512],
                            fT[:, 2 * tp : 2 * tp + 2, 0:128],
                            kv_sb[
                                :,
                                2 * tp : 2 * tp + 2,
                                (2 * k + 1) * 512 : (2 * k + 2) * 512,
                            ],
                            start=st,
                            stop=sp,
                            perf_mode=PM.DoubleRow,
                        )
                    mA.then_inc(s_p, 1)
                    mB.then_inc(s_p, 1)

                def p2triple(r):
                    pe.wait_ge(s_a, TRIP_SA[r])
                    if TRIP_SV[r]:
                        pe.wait_ge(s_v, TRIP_SV[r])
                    for pc in TRIP_KV[r]:
                        pe.wait_ge(s_kv[pc], 16)
                    ng = 3 if r < 3 else 1
                    mfin = [None] * ng
                    for t6 in range(6):
                        st, sp = t6 == 0, t6 == 5
                        for g in range(ng):
                            mfin[g] = nc.tensor.matmul(
                                RB[g][32 * g : 32 * g + 32, 0:512],
                                fT[:, t6, 128:160],
                                kv_sb[:, t6, (3 * r + g) * 512 : (3 * r + g + 1) * 512],
                                start=st,
                                stop=sp,
                            )
                    for g in range(ng):
                        mfin[g].then_inc(s_p, 1)

                # interleave m0 pairs and m1 triples so drains overlap compute
                p2pair(0)     # s_p 18, 19
                p2triple(0)   # s_p 20, 21, 22
                p2pair(1)     # s_p 23, 24
                p2triple(1)   # s_p 25, 26, 27
                p2pair(2)     # s_p 28, 29
                p2triple(2)   # s_p 30, 31, 32
                p2pair(3)     # s_p 33, 34
                p2triple(3)   # s_p 35
                p2pair(4)     # s_p 36, 37

            @block.scalar
            def _(act):
                # warm the exp table set during the DMA window
                act.wait_ge(s_z, 16)
                nc.scalar.activation(
                    scr[:, :], zz_sb[:, 0, 0:8], AF.Exp, bias=0.0, scale=1.0
                )
                act.wait_ge(s_i, 16)
                # e64 = 64 * exp(s * SCALE); bias AP holds ln(64)
                for g in range(3):
                    act.wait_ge(s_p, 1 + g)
                    nc.scalar.activation(
                        e_lo[32 * g : 32 * g + 32, 0:256],
                        M1B[g][32 * g : 32 * g + 32, 0:256],
                        AF.Exp,
                        bias=idc[32 * g : 32 * g + 32, 128:129],
                        scale=SCALE,
                    ).then_inc(s_a, 1)  # 1, 2, 3
                act.wait_ge(s_p, 4)
                nc.scalar.activation(
                    e_hi[:, 512:768], ps_b[0:128, 0:256], AF.Exp,
                    bias=idc[:, 128:129], scale=SCALE,
                ).then_inc(s_a, 1)  # 4
                act.wait_ge(s_p, 5)
                nc.scalar.activation(
                    e_hi[:, 0:512], ps_a[0:128, 0:512], AF.Exp,
                    bias=idc[:, 128:129], scale=SCALE,
                ).then_inc(s_a, 1)  # 5
                # phase-2 m1 drains (gA, gB per triple; DVE takes gC)
                for r in range(4):
                    for gi in range(2 if r < 3 else 1):
                        act.wait_ge(s_p, TRIP_SP[r][gi])
                        nc.scalar.activation(
                            o_lo[32 * gi : 32 * gi + 32, r, :],
                            RB[gi][32 * gi : 32 * gi + 32, 0:512],
                            AF.Copy,
                        ).then_inc(s_a, 1)  # 6..12

            @block.vector
            def _(dve):
                dve.wait_ge(s_a, 3)
                nc.vector.reduce_sum(
                    dl_sb[:], e_lo[:, :], axis=mybir.AxisListType.X
                ).then_inc(s_v, 1)  # 1
                # fT copies: f64 = e64T - 64, cast to fp8
                for k in range(6):
                    dve.wait_ge(s_p, 6 + k)
                    nc.vector.tensor_scalar_sub(
                        fT[:, k, 128:160], TR1_BANK[k][0:128, 0:32], 64.0
                    ).then_inc(s_v, 1)  # 2..7
                for k in range(6):
                    dve.wait_ge(s_p, 12 + k)
                    nc.vector.tensor_scalar_sub(
                        fT[:, TR0_CHUNK[k], 0:128], TR0_BANK[k][0:128, 0:128], 64.0
                    ).then_inc(s_v, 1)  # 8..13
                dve.wait_ge(s_a, 5)
                nc.vector.reduce_sum(
                    dh_sb[:], e_hi[:, :], axis=mybir.AxisListType.X
                ).then_inc(s_v, 1)  # 14
                # phase-2 drains: m0 n-tiles to o_hi, m1 gC rows to o_lo
                def nt_drain(g, sp_val, bank):
                    dve.wait_ge(s_p, sp_val)
                    nc.vector.tensor_copy(
                        o_hi[:, g * 512 : (g + 1) * 512], bank[0:128, 0:512]
                    ).then_inc(s_v, 1)

                def gc_drain(r, sp_val):
                    dve.wait_ge(s_p, sp_val)
                    nc.vector.tensor_copy(
                        o_lo[64:96, r, :], ps_m[64:96, 0:512]
                    ).then_inc(s_v, 1)

                nt_drain(0, 18, ps_t[2])   # 15
                nt_drain(1, 19, ps_t[3])   # 16
                gc_drain(0, 22)            # 17
                nt_drain(2, 23, ps_t[0])   # 18
                nt_drain(3, 24, ps_t[1])   # 19
                gc_drain(1, 27)            # 20
                nt_drain(4, 28, ps_t[2])   # 21
                nt_drain(5, 29, ps_t[3])   # 22
                gc_drain(2, 32)            # 23
                nt_drain(6, 33, ps_t[0])   # 24
                nt_drain(7, 34, ps_t[1])   # 25
                nt_drain(8, 36, ps_t[2])   # 26
                nt_drain(9, 37, ps_t[3])   # 27

    return nc


def _host_prep(z1_hat, z2):
    z1 = np.asarray(z1_hat, dtype=np.float32)[0]  # [128, 100, 64]
    z2a = np.asarray(z2, dtype=np.float32)[0]

    # q patches [160, 5120]; device layout q8 [128, j, i, 160]
    q = z1.reshape(KC, NH, KH, NW, KW).transpose(1, 3, 0, 2, 4).reshape(PQ, D)
    q4 = q.reshape(PQ, KC, KH, KW)
    q8 = q4.transpose(1, 3, 2, 0).astype(F8)  # [c, j, i, p]
    qm0 = np.ascontiguousarray(q8[:, :, :, 0:128])
    qm1 = np.ascontiguousarray(q8[:, :, :, 128:160])

    # padded z2: rows 100..111 zero
    z_pad = np.zeros((KC, 112, W), dtype=np.float32)
    z_pad[:, :H] = z2a
    z8_pad = z_pad.astype(F8)

    # sliding kv patches from padded z2 (original fp32 values, cast per-row)
    sw = np.lib.stride_tricks.sliding_window_view(z_pad, (KH, KW), axis=(1, 2))

    q64 = q.astype(np.float64)
    ij_off = (np.arange(KH)[:, None] * W + np.arange(KW)[None, :]).reshape(-1)  # [40]

    idc = np.zeros((128, 129), dtype=np.float32)
    idc[:, 0:128] = np.eye(128, dtype=np.float32)
    idc[:, 128] = LN64

    in_maps = []
    corrs = []
    for core in range(NCORES):
        h0 = HPC * core
        slab8 = z8_pad[:, h0 : h0 + ZROWS, :].reshape(KC, ZLEN)
        zz = np.zeros((KC, 2, ZLEN), dtype=F8)
        zz[:, 0, :] = slab8
        zz[:, 1, 0 : ZLEN - W] = slab8[:, W:]
        # kv rows indexed by flat position p = h_local*64 + w
        kvp = np.zeros((PKC, D), dtype=F8)
        hh = np.arange(PKC) // W
        ww = np.arange(PKC) % W
        real = (ww < WK) & (h0 + hh < HK)
        ridx = np.nonzero(real)[0]
        kvp[ridx] = (
            sw[:, h0 + hh[ridx], ww[ridx]].transpose(1, 0, 2, 3).reshape(-1, D)
        ).astype(F8)
        kv8 = np.ascontiguousarray(kvp.reshape(6, 128, D).transpose(1, 0, 2))
        in_maps.append(
            {"zz": zz, "qm0": qm0, "qm1": qm1, "kv8": kv8, "idc": idc}
        )
        # denominator correction: computed-but-invalid stream positions,
        # recomputed exactly (fp64) from the original values.
        bad = np.nonzero(~real)[0]
        zf = z_pad[:, h0 : h0 + ZROWS, :].reshape(KC, ZLEN).astype(np.float64)
        win = zf[:, bad[:, None] + ij_off[None, :]]  # [128, nb, 40]
        patches = win.transpose(1, 0, 2).reshape(len(bad), D)
        s_bad = q64 @ patches.T  # [160, nb]
        corrs.append(np.exp(s_bad * SCALE).sum(axis=1))

    corr = np.sum(corrs, axis=0)
    swr = sw[:, :HK, :WK]
    colsum = swr.astype(np.float64).sum(axis=(1, 2)).reshape(D)  # [5120]
    return in_maps, corr, colsum


def kernel(z1_hat, z2):
    from concourse.bass_utils import run_bass_kernel_spmd

    in_maps, corr, colsum = _host_prep(z1_hat, z2)
    if "nc" not in _CACHE:
        _CACHE["nc"] = _build_nc()
    nc = _CACHE["nc"]
    res = run_bass_kernel_spmd(nc, in_maps, list(range(NCORES)))
    num = np.broadcast_to(colsum, (PQ, D)).astype(np.float64).copy()
    den = -corr
    for r in res.results:
        ohi = r["ohi"].astype(np.float64)  # [128, 5120] = 64 * partial m0
        olo = r["olo"].astype(np.float64).reshape(96, 4, 512)
        part = np.zeros((PQ, D))
        part[0:128] = ohi
        for nt in range(10):
            rr, g = nt // 3, nt % 3
            part[128:160, nt * 512 : (nt + 1) * 512] = olo[32 * g : 32 * g + 32, rr]
        num += part / 64.0
        dv = r["den"].astype(np.float64)[:, 0] / 64.0
        den = den + np.concatenate(
            [dv[0:128], dv[128:160] + dv[160:192] + dv[192:224]]
        )
    out = (num / den[:, None]).astype(np.float32)
    # fold patches back: [160, 5120] -> [1, 128, 100, 64]
    out = out.reshape(NH, NW, KC, KH, KW).transpose(2, 0, 3, 1, 4)
    return np.ascontiguousarray(out.reshape(1, KC, H, W))


# revision 14
# speedup vs baseline: 1.3723x; 1.0154x over previous
"""Trainium2 Bass kernel for BottleneckAttention (patch attention), fp8 edition.

q patches [160, 5120] from z1_hat (non-overlapping 10x4 unfold),
kv patches [5551, 5120] from z2 (overlapping unfold, Hk=91 x Wk=61),
scores = q @ kv.T / 5120, softmax over kv patches, out = attn @ kv,
folded back to [1, 128, 100, 64].

Sharding: contiguous blocks of 12 kv h-rows per core (8 x 12 = 96 >= 91).
Each core owns the 768 flat positions p = h_local*64 + w (w in [0,64));
positions with w >= 61 or h >= 91 are invalid -- their kv rows are zeroed
so they never touch the numerator, and the host subtracts their exactly
recomputed exp contribution from the denominator. Every core computes all
160 q rows; the host combines with an all-gather softmax.

Per-core kernel (raw Bass, explicit semaphores), fp8e4m3 everywhere on
the PE with DoubleRow (K=256) perf mode for the M=128 matmul blocks:
  phase 1: scores as implicit convolution against the SBUF-resident
    z2 slab (zz holds the slab plus a 64-shifted copy so (i, i+1) kernel
    row pairs form clean [128, 2, N] DoubleRow moving operands).
    q rows 0..127 run M=128 DoubleRow; rows 128..159 run as three
    concurrent 32-wide PE column groups (no DoubleRow -- col tiling and
    DoubleRow are mutually exclusive).
  exp on ScalarE with scale=1/5120 and bias=ln(64): e64 = 64*exp(s).
  row-sum denominator (64x) on VectorE; host divides by 64.
  PE transposes of e64 chunks; the ACT psum->sbuf copy applies bias=-64
  so the fp8 fT stores f64 = 64*(e-1) (centered softmax, scaled into
  fp8e4m3's normal range; the host adds the exact sum-of-kv-columns term
  and divides by 64).
  phase 2: partial_out = f64T.T @ kv_shard in fp8 DoubleRow (m0) plus
    three-column-group fp8 (m1), kv resident in SBUF, drained to bf16.
"""

import sys

sys.path.insert(0, "/opt/trn_rl_repo")

import numpy as np
import ml_dtypes

import concourse.bass as bass
import concourse.mybir as mybir

DT = mybir.dt
AF = mybir.ActivationFunctionType
PM = mybir.MatmulPerfMode

# problem geometry (hardcoded from the reference module)
KC, KH, KW = 128, 10, 4
H, W = 100, 64
NH, NW = H // KH, W // KW          # 10, 16
PQ = NH * NW                       # 160 q patches
D = KC * KH * KW                   # 5120
HK, WK = H - KH + 1, W - KW + 1    # 91, 61
NCORES = 8
HPC = 12                           # kv h-rows per core (8*12 = 96 >= 91)
PKC = HPC * W                      # 768 flat positions per core
ZROWS = 2 * HPC                    # 24 z rows staged per core
ZLEN = ZROWS * W                   # 1536
SCALE = 1.0 / D
LN64 = float(np.log(64.0))
F8 = ml_dtypes.float8_e4m3fn

_CACHE = {}


def _build_nc():
    nc = bass.Bass()
    zz_d = nc.declare_dram_parameter("zz", [KC, 2, ZLEN], DT.float8e4, isOutput=False)
    qm1_d = nc.declare_dram_parameter("qm1", [KC, KW, KH, 32], DT.float8e4, isOutput=False)
    idc_d = nc.declare_dram_parameter("idc", [128, 129], DT.float32, isOutput=False)
    qm0_d = nc.declare_dram_parameter("qm0", [KC, KW, KH, 128], DT.float8e4, isOutput=False)
    kv_d = nc.declare_dram_parameter("kv8", [128, 6, D], DT.float8e4, isOutput=False)
    ohi_d = nc.declare_dram_parameter("ohi", [128, D], DT.bfloat16, isOutput=True)
    olo_d = nc.declare_dram_parameter("olo", [96, 4, 512], DT.bfloat16, isOutput=True)
    den_d = nc.declare_dram_parameter("den", [224, 1], DT.float32, isOutput=True)

    from contextlib import ExitStack

    ctx = ExitStack()
    with ctx:
        zz_sb = ctx.enter_context(nc.sbuf_tensor([KC, 2, ZLEN], DT.float8e4))
        qm1_sb = ctx.enter_context(nc.sbuf_tensor([KC, KW, KH, 32], DT.float8e4))
        qm0_sb = ctx.enter_context(nc.sbuf_tensor([KC, KW, KH, 128], DT.float8e4))
        idc = ctx.enter_context(nc.sbuf_tensor([128, 129], DT.float32))
        kv_sb = ctx.enter_context(nc.sbuf_tensor([128, 6, D], DT.float8e4))
        e_hi = ctx.enter_context(nc.sbuf_tensor([128, PKC], DT.float32))
        e_lo = ctx.enter_context(nc.sbuf_tensor([96, 256], DT.float32))
        fT = ctx.enter_context(nc.sbuf_tensor([128, 6, PQ], DT.float8e4))
        o_hi = ctx.enter_context(nc.sbuf_tensor([128, D], DT.bfloat16))
        o_lo = ctx.enter_context(nc.sbuf_tensor([96, 4, 512], DT.bfloat16))
        dh_sb = ctx.enter_context(nc.sbuf_tensor([128, 1], DT.float32))
        dl_sb = ctx.enter_context(nc.sbuf_tensor([96, 1], DT.float32))
        scr = ctx.enter_context(nc.sbuf_tensor([128, 8], DT.float32))
        wz = ctx.enter_context(nc.sbuf_tensor([128, 512], DT.float8e4))

        ps_a = ctx.enter_context(nc.psum_tensor("ps_a", [128, 512], DT.float32))
        ps_b = ctx.enter_context(nc.psum_tensor("ps_b", [128, 512], DT.float32))
        ps_m = ctx.enter_context(nc.psum_tensor("ps_m", [128, 512], DT.float32))
        ps_w = ctx.enter_context(nc.psum_tensor("ps_w", [128, 512], DT.float32))
        ps_t = [
            ctx.enter_context(nc.psum_tensor(f"ps_t{i}", [128, 512], DT.float32))
            for i in range(4)
        ]

        s_z = ctx.enter_context(nc.semaphore("s_z"))
        s_q1 = ctx.enter_context(nc.semaphore("s_q1"))
        s_q0 = ctx.enter_context(nc.semaphore("s_q0"))
        s_i = ctx.enter_context(nc.semaphore("s_i"))
        s_kv = [ctx.enter_context(nc.semaphore(f"s_kv{i}")) for i in range(3)]
        s_p = ctx.enter_context(nc.semaphore("s_p"))
        s_a = ctx.enter_context(nc.semaphore("s_a"))
        s_v = ctx.enter_context(nc.semaphore("s_v"))
        s_o = ctx.enter_context(nc.semaphore("s_o"))
        s_g = ctx.enter_context(nc.semaphore("s_g"))

        # p1 m1 col-groups: gA->ps_m[0:32], gB->ps_t0[32:64], gC->ps_t1[64:96]
        M1B = [ps_m, ps_t[0], ps_t[1]]
        # TR1 transposes (m1 e_lo chunks 0..5) alternate banks t2/t3
        TR1_BANK = [ps_t[2], ps_t[3], ps_t[2], ps_t[3], ps_t[2], ps_t[3]]
        TR1_SA = [1, 1, 2, 2, 3, 3]       # e_lo chunk needs its group's exp
        TR1_SV = [0, 0, 2, 3, 4, 5]       # bank drain (DVE fT copy) for reuse
        # TR0 emission order: e_hi chunks [4,5,0,1,2,3] on banks [t0,t1,t2,t3,t0,t1]
        TR0_CHUNK = [4, 5, 0, 1, 2, 3]
        TR0_BANK = [ps_t[0], ps_t[1], ps_t[2], ps_t[3], ps_t[0], ps_t[1]]
        TR0_SA = [4, 4, 5, 5, 5, 5]       # chunks 4,5 <- exp B; 0..3 <- exp A
        TR0_SV = [0, 0, 6, 7, 9, 10]
        # phase-2 m1 triples (n-tiles 3r..3r+2) on banks (w, b, m)
        RB = [ps_w, ps_b, ps_m]
        TRIP_SA = [4, 7, 9, 10]
        TRIP_SV = [7, 8, 16, 0]
        TRIP_KV = [[0], [1], [1, 2], [2]]
        TRIP_SP = [(12, 13, 14), (21, 22, 23), (26, 27, 28), (31,)]

        with nc.Block() as block:

            @block.sync
            def _(sync):
                sync.dma_start(zz_sb[:, :, :], zz_d[:]).then_inc(s_z, 16)
                sync.dma_start(qm1_sb[:, :, :, :], qm1_d[:]).then_inc(s_q1, 16)
                sync.dma_start(idc[:, :], idc_d[:]).then_inc(s_i, 16)
                sync.dma_start(qm0_sb[:, :, :, :], qm0_d[:]).then_inc(s_q0, 16)
                sync.dma_start(kv_sb[:, :, 0:1536], kv_d[:, :, 0:1536]).then_inc(
                    s_kv[0], 16
                )
                sync.dma_start(kv_sb[:, :, 1536:3584], kv_d[:, :, 1536:3584]).then_inc(
                    s_kv[1], 16
                )
                sync.dma_start(kv_sb[:, :, 3584:5120], kv_d[:, :, 3584:5120]).then_inc(
                    s_kv[2], 16
                )
                sync.wait_ge(s_v, 1)
                sync.dma_start(den_d[128:224, :], dl_sb[:]).then_inc(s_o, 16)
                sync.wait_ge(s_a, 7)
                sync.wait_ge(s_v, 8)
                sync.dma_start(olo_d[:, 0, :], o_lo[:, 0, :]).then_inc(s_o, 16)
                sync.wait_ge(s_v, 15)
                sync.dma_start(den_d[0:128, :], dh_sb[:]).then_inc(s_o, 16)
                sync.wait_ge(s_a, 9)
                sync.wait_ge(s_v, 16)
                sync.dma_start(olo_d[:, 1, :], o_lo[:, 1, :]).then_inc(s_o, 16)
                sync.wait_ge(s_v, 18)
                sync.dma_start(ohi_d[:, 0:1024], o_hi[:, 0:1024]).then_inc(s_o, 16)
                sync.wait_ge(s_a, 11)
                sync.wait_ge(s_v, 19)
                sync.dma_start(olo_d[:, 2, :], o_lo[:, 2, :]).then_inc(s_o, 16)
                sync.wait_ge(s_v, 21)
                sync.dma_start(ohi_d[:, 1024:2048], o_hi[:, 1024:2048]).then_inc(
                    s_o, 16
                )
                sync.wait_ge(s_a, 12)
                sync.dma_start(olo_d[0:32, 3, :], o_lo[0:32, 3, :]).then_inc(s_o, 16)
                sync.wait_ge(s_v, 23)
                sync.dma_start(ohi_d[:, 2048:3072], o_hi[:, 2048:3072]).then_inc(
                    s_o, 16
                )
                sync.wait_ge(s_v, 25)
                sync.dma_start(ohi_d[:, 3072:4096], o_hi[:, 3072:4096]).then_inc(
                    s_o, 16
                )
                sync.wait_ge(s_v, 27)
                sync.dma_start(ohi_d[:, 4096:5120], o_hi[:, 4096:5120]).then_inc(
                    s_o, 16
                )
                sync.wait_ge(s_o, 176)

            @block.tensor
            def _(pe):
                # HAM warmup on the DVE-memset tile: PE busy from ~4us with no
                # DMA dependency, so phase 1 starts at the warm 2.4 GHz clock.
                pe.wait_ge(s_g, 1)
                for w in range(20):
                    nc.tensor.matmul(
                        ps_w[0:128, 0:512],
                        wz[:, 0:128],
                        wz[:, 0:512],
                        start=(w == 0),
                        stop=(w == 19),
                    )
                pe.wait_ge(s_z, 16)
                pe.wait_ge(s_q1, 16)
                # phase 1 m1 (q rows 128..159): 3 concurrent 32-col groups
                mfin = [None, None, None]
                for i in range(KH):
                    for j in range(KW):
                        st = i == 0 and j == 0
                        sp = i == KH - 1 and j == KW - 1
                        off = i * W + j
                        for g in range(3):
                            mfin[g] = nc.tensor.matmul(
                                M1B[g][32 * g : 32 * g + 32, 0:256],
                                qm1_sb[:, j, i, :],
                                zz_sb[:, 0, off + 256 * g : off + 256 * g + 256],
                                start=st,
                                stop=sp,
                            )
                for g in range(3):
                    mfin[g].then_inc(s_p, 1)  # s_p = 1, 2, 3
                pe.wait_ge(s_q0, 16)
                # phase 1 m0 chain B (pos 512:768), DoubleRow (i, i+1) pairs
                for j in range(KW):
                    for ip in range(5):
                        st = j == 0 and ip == 0
                        sp = j == KW - 1 and ip == 4
                        off = (2 * ip) * W + j
                        mm = nc.tensor.matmul(
                            ps_b[0:128, 0:256],
                            qm0_sb[:, j, 2 * ip : 2 * ip + 2, :],
                            zz_sb[:, :, off + 512 : off + 768],
                            start=st,
                            stop=sp,
                            perf_mode=PM.DoubleRow,
                        )
                mm.then_inc(s_p, 1)  # s_p = 4
                # TR1: transpose e_lo (m1) chunks -> banks t2/t3 (exp B and
                # chain A hide the ACT/DVE latency around these)
                pe.wait_ge(s_i, 16)
                for k in range(6):
                    g = k // 2
                    col = (k % 2) * 128
                    pe.wait_ge(s_a, TR1_SA[k])
                    if TR1_SV[k]:
                        pe.wait_ge(s_v, TR1_SV[k])
                    nc.tensor.matmul(
                        TR1_BANK[k][0:128, 0:32],
                        e_lo[32 * g : 32 * g + 32, col : col + 128],
                        idc[32 * g : 32 * g + 32, 32 * g : 32 * g + 32],
                        is_transpose=True,
                        start=True,
                        stop=True,
                    ).then_inc(s_p, 1)  # s_p = 5..10
                # phase 1 m0 chain A (pos 0:512)
                for j in range(KW):
                    for ip in range(5):
                        st = j == 0 and ip == 0
                        sp = j == KW - 1 and ip == 4
                        off = (2 * ip) * W + j
                        mm = nc.tensor.matmul(
                            ps_a[0:128, 0:512],
                            qm0_sb[:, j, 2 * ip : 2 * ip + 2, :],
                            zz_sb[:, :, off : off + 512],
                            start=st,
                            stop=sp,
                            perf_mode=PM.DoubleRow,
                        )
                mm.then_inc(s_p, 1)  # s_p = 11

                def p2triple(r, sp_base):
                    pe.wait_ge(s_a, TRIP_SA[r])
                    if TRIP_SV[r]:
                        pe.wait_ge(s_v, TRIP_SV[r])
                    for pc in TRIP_KV[r]:
                        pe.wait_ge(s_kv[pc], 16)
                    ng = 3 if r < 3 else 1
                    mf = [None] * ng
                    for t6 in range(6):
                        st, sp = t6 == 0, t6 == 5
                        for g in range(ng):
                            mf[g] = nc.tensor.matmul(
                                RB[g][32 * g : 32 * g + 32, 0:512],
                                fT[:, t6, 128:160],
                                kv_sb[:, t6, (3 * r + g) * 512 : (3 * r + g + 1) * 512],
                                start=st,
                                stop=sp,
                            )
                    for g in range(ng):
                        mf[g].then_inc(s_p, 1)

                # r0 runs in the exp-A latency shadow right after chain A
                p2triple(0, 12)  # s_p = 12, 13, 14
                # TR0: transpose e_hi chunks, exp-B-dependent chunks first
                for k in range(6):
                    c = TR0_CHUNK[k]
                    pe.wait_ge(s_a, TR0_SA[k])
                    if TR0_SV[k]:
                        pe.wait_ge(s_v, TR0_SV[k])
                    nc.tensor.matmul(
                        TR0_BANK[k][0:128, 0:128],
                        e_hi[:, c * 128 : (c + 1) * 128],
                        idc[0:128, 0:128],
                        is_transpose=True,
                        start=True,
                        stop=True,
                    ).then_inc(s_p, 1)  # s_p = 15..20

                def p2pair(k, banks, sv, kvs):
                    pe.wait_ge(s_v, sv)
                    for pc in kvs:
                        pe.wait_ge(s_kv[pc], 16)
                    bA, bB = ps_t[banks[0]], ps_t[banks[1]]
                    for tp in range(3):
                        st, sp = tp == 0, tp == 2
                        mA = nc.tensor.matmul(
                            bA[0:128, 0:512],
                            fT[:, 2 * tp : 2 * tp + 2, 0:128],
                            kv_sb[
                                :, 2 * tp : 2 * tp + 2, (2 * k) * 512 : (2 * k + 1) * 512
                            ],
                            start=st,
                            stop=sp,
                            perf_mode=PM.DoubleRow,
                        )
                        mB = nc.tensor.matmul(
                            bB[0:128, 0:512],
                            fT[:, 2 * tp : 2 * tp + 2, 0:128],
                            kv_sb[
                                :,
                                2 * tp : 2 * tp + 2,
                                (2 * k + 1) * 512 : (2 * k + 2) * 512,
                            ],
                            start=st,
                            stop=sp,
                            perf_mode=PM.DoubleRow,
                        )
                    mA.then_inc(s_p, 1)
                    mB.then_inc(s_p, 1)

                p2triple(1, 21)              # s_p = 21, 22, 23
                p2pair(0, (2, 3), 14, [0])   # s_p = 24, 25
                p2triple(2, 26)              # s_p = 26, 27, 28
                p2pair(1, (0, 1), 14, [0, 1])  # s_p = 29, 30
                p2triple(3, 31)              # s_p = 31
                p2pair(2, (2, 3), 18, [1])     # s_p = 32, 33
                p2pair(3, (0, 1), 21, [1, 2])  # s_p = 34, 35
                p2pair(4, (2, 3), 23, [2])     # s_p = 36, 37

            @block.scalar
            def _(act):
                # warm the exp table set early (reads the memset tile)
                act.wait_ge(s_g, 1)
                nc.scalar.activation(
                    scr[:, :], wz[:, 0:8], AF.Exp, bias=0.0, scale=1.0
                )
                act.wait_ge(s_i, 16)
                # e64 = 64 * exp(s * SCALE); bias AP holds ln(64)
                for g in range(3):
                    act.wait_ge(s_p, 1 + g)
                    nc.scalar.activation(
                        e_lo[32 * g : 32 * g + 32, 0:256],
                        M1B[g][32 * g : 32 * g + 32, 0:256],
                        AF.Exp,
                        bias=idc[32 * g : 32 * g + 32, 128:129],
                        scale=SCALE,
                    ).then_inc(s_a, 1)  # 1, 2, 3
                act.wait_ge(s_p, 4)
                nc.scalar.activation(
                    e_hi[:, 512:768], ps_b[0:128, 0:256], AF.Exp,
                    bias=idc[:, 128:129], scale=SCALE,
                ).then_inc(s_a, 1)  # 4
                act.wait_ge(s_p, 11)
                nc.scalar.activation(
                    e_hi[:, 0:512], ps_a[0:128, 0:512], AF.Exp,
                    bias=idc[:, 128:129], scale=SCALE,
                ).then_inc(s_a, 1)  # 5
                # phase-2 m1 drains (gA, gB per triple; DVE takes gC)
                for r in range(4):
                    for gi in range(2 if r < 3 else 1):
                        act.wait_ge(s_p, TRIP_SP[r][gi])
                        nc.scalar.activation(
                            o_lo[32 * gi : 32 * gi + 32, r, :],
                            RB[gi][32 * gi : 32 * gi + 32, 0:512],
                            AF.Copy,
                        ).then_inc(s_a, 1)  # 6..12

            @block.vector
            def _(dve):
                nc.vector.memset(wz[:, :], 0.0).then_inc(s_g, 1)
                dve.wait_ge(s_a, 3)
                nc.vector.reduce_sum(
                    dl_sb[:], e_lo[:, :], axis=mybir.AxisListType.X
                ).then_inc(s_v, 1)  # 1
                # fT copies: f64 = e64T - 64, cast to fp8
                for k in range(6):
                    dve.wait_ge(s_p, 5 + k)
                    nc.vector.tensor_scalar_sub(
                        fT[:, k, 128:160], TR1_BANK[k][0:128, 0:32], 64.0
                    ).then_inc(s_v, 1)  # 2..7

                def gc_drain(r, sp_val):
                    dve.wait_ge(s_p, sp_val)
                    nc.vector.tensor_copy(
                        o_lo[64:96, r, :], ps_m[64:96, 0:512]
                    ).then_inc(s_v, 1)

                gc_drain(0, 14)            # 8
                for k in range(6):
                    dve.wait_ge(s_p, 15 + k)
                    nc.vector.tensor_scalar_sub(
                        fT[:, TR0_CHUNK[k], 0:128], TR0_BANK[k][0:128, 0:128], 64.0
                    ).then_inc(s_v, 1)  # 9..14
                dve.wait_ge(s_a, 5)
                nc.vector.reduce_sum(
                    dh_sb[:], e_hi[:, :], axis=mybir.AxisListType.X
                ).then_inc(s_v, 1)  # 15

                def nt_drain(g, sp_val, bank):
                    dve.wait_ge(s_p, sp_val)
                    nc.vector.tensor_copy(
                        o_hi[:, g * 512 : (g + 1) * 512], bank[0:128, 0:512]
                    ).then_inc(s_v, 1)

                gc_drain(1, 23)            # 16
                nt_drain(0, 24, ps_t[2])   # 17
                nt_drain(1, 25, ps_t[3])   # 18
                gc_drain(2, 28)            # 19
                nt_drain(2, 29, ps_t[0])   # 20
                nt_drain(3, 30, ps_t[1])   # 21
                nt_drain(4, 32, ps_t[2])   # 22
                nt_drain(5, 33, ps_t[3])   # 23
                nt_drain(6, 34, ps_t[0])   # 24
                nt_drain(7, 35, ps_t[1])   # 25
                nt_drain(8, 36, ps_t[2])   # 26
                nt_drain(9, 37, ps_t[3])   # 27

    return nc


def _host_prep(z1_hat, z2):
    z1 = np.asarray(z1_hat, dtype=np.float32)[0]  # [128, 100, 64]
    z2a = np.asarray(z2, dtype=np.float32)[0]

    # q patches [160, 5120]; device layout q8 [128, j, i, 160]
    q = z1.reshape(KC, NH, KH, NW, KW).transpose(1, 3, 0, 2, 4).reshape(PQ, D)
    q4 = q.reshape(PQ, KC, KH, KW)
    q8 = q4.transpose(1, 3, 2, 0).astype(F8)  # [c, j, i, p]
    qm0 = np.ascontiguousarray(q8[:, :, :, 0:128])
    qm1 = np.ascontiguousarray(q8[:, :, :, 128:160])

    # padded z2: rows 100..111 zero
    z_pad = np.zeros((KC, 112, W), dtype=np.float32)
    z_pad[:, :H] = z2a
    z8_pad = z_pad.astype(F8)

    # sliding kv patches from padded z2 (original fp32 values, cast per-row)
    sw = np.lib.stride_tricks.sliding_window_view(z_pad, (KH, KW), axis=(1, 2))

    q64 = q.astype(np.float64)
    ij_off = (np.arange(KH)[:, None] * W + np.arange(KW)[None, :]).reshape(-1)  # [40]

    idc = np.zeros((128, 129), dtype=np.float32)
    idc[:, 0:128] = np.eye(128, dtype=np.float32)
    idc[:, 128] = LN64

    in_maps = []
    corrs = []
    for core in range(NCORES):
        h0 = HPC * core
        slab8 = z8_pad[:, h0 : h0 + ZROWS, :].reshape(KC, ZLEN)
        zz = np.zeros((KC, 2, ZLEN), dtype=F8)
        zz[:, 0, :] = slab8
        zz[:, 1, 0 : ZLEN - W] = slab8[:, W:]
        # kv rows indexed by flat position p = h_local*64 + w
        kvp = np.zeros((PKC, D), dtype=F8)
        hh = np.arange(PKC) // W
        ww = np.arange(PKC) % W
        real = (ww < WK) & (h0 + hh < HK)
        ridx = np.nonzero(real)[0]
        kvp[ridx] = (
            sw[:, h0 + hh[ridx], ww[ridx]].transpose(1, 0, 2, 3).reshape(-1, D)
        ).astype(F8)
        kv8 = np.ascontiguousarray(kvp.reshape(6, 128, D).transpose(1, 0, 2))
        in_maps.append(
            {"zz": zz, "qm0": qm0, "qm1": qm1, "kv8": kv8, "idc": idc}
        )
        # denominator correction: computed-but-invalid stream positions,
        # recomputed exactly (fp64) from the original values.
        bad = np.nonzero(~real)[0]
        zf = z_pad[:, h0 : h0 + ZROWS, :].reshape(KC, ZLEN).astype(np.float64)
        win = zf[:, bad[:, None] + ij_off[None, :]]  # [128, nb, 40]
        patches = win.transpose(1, 0, 2).reshape(len(bad), D)
        s_bad = q64 @ patches.T  # [160, nb]
        corrs.append(np.exp(s_bad * SCALE).sum(axis=1))

    corr = np.sum(corrs, axis=0)
    swr = sw[:, :HK, :WK]
    colsum = swr.astype(np.float64).sum(axis=(1, 2)).reshape(D)  # [5120]
    return in_maps, corr, colsum


def kernel(z1_hat, z2):
    from concourse.bass_utils import run_bass_kernel_spmd

    in_maps, corr, colsum = _host_prep(z1_hat, z2)
    if "nc" not in _CACHE:
        _CACHE["nc"] = _build_nc()
    nc = _CACHE["nc"]
    res = run_bass_kernel_spmd(nc, in_maps, list(range(NCORES)))
    num = np.broadcast_to(colsum, (PQ, D)).astype(np.float64).copy()
    den = -corr
    for r in res.results:
        ohi = r["ohi"].astype(np.float64)  # [128, 5120] = 64 * partial m0
        olo = r["olo"].astype(np.float64).reshape(96, 4, 512)
        part = np.zeros((PQ, D))
        part[0:128] = ohi
        for nt in range(10):
            rr, g = nt // 3, nt % 3
            part[128:160, nt * 512 : (nt + 1) * 512] = olo[32 * g : 32 * g + 32, rr]
        num += part / 64.0
        dv = r["den"].astype(np.float64)[:, 0] / 64.0
        den = den + np.concatenate(
            [dv[0:128], dv[128:160] + dv[160:192] + dv[192:224]]
        )
    out = (num / den[:, None]).astype(np.float32)
    # fold patches back: [160, 5120] -> [1, 128, 100, 64]
    out = out.reshape(NH, NW, KC, KH, KW).transpose(2, 0, 3, 1, 4)
    return np.ascontiguousarray(out.reshape(1, KC, H, W))


# revision 16
# speedup vs baseline: 1.4826x; 1.0803x over previous
"""Trainium2 Bass kernel for BottleneckAttention (patch attention), fp8 edition.

q patches [160, 5120] from z1_hat (non-overlapping 10x4 unfold),
kv patches [5551, 5120] from z2 (overlapping unfold, Hk=91 x Wk=61),
scores = q @ kv.T / 5120, softmax over kv patches, out = attn @ kv,
folded back to [1, 128, 100, 64].

Sharding: contiguous blocks of 12 kv h-rows per core (8 x 12 = 96 >= 91).
Each core owns the 768 flat positions p = h_local*64 + w (w in [0,64));
positions with w >= 61 or h >= 91 are invalid -- their kv rows are zeroed
so they never touch the numerator, and the host subtracts their exactly
recomputed exp contribution from the denominator. Every core computes all
160 q rows; the host combines with an all-gather softmax.

Per-core kernel (raw Bass, explicit semaphores), fp8e4m3 everywhere on
the PE with DoubleRow (K=256) perf mode for the M=128 matmul blocks:
  phase 1: scores as implicit convolution against the SBUF-resident
    z2 slab (zz holds the slab plus a 64-shifted copy so (i, i+1) kernel
    row pairs form clean [128, 2, N] DoubleRow moving operands).
    q rows 0..127 run M=128 DoubleRow; rows 128..159 run as three
    concurrent 32-wide PE column groups (no DoubleRow -- col tiling and
    DoubleRow are mutually exclusive).
  exp on ScalarE with scale=1/5120 and bias=ln(64): e64 = 64*exp(s).
  row-sum denominator (64x) on VectorE; host divides by 64.
  PE transposes of e64 chunks; the ACT psum->sbuf copy applies bias=-64
  so the fp8 fT stores f64 = 64*(e-1) (centered softmax, scaled into
  fp8e4m3's normal range; the host adds the exact sum-of-kv-columns term
  and divides by 64).
  phase 2: partial_out = f64T.T @ kv_shard in fp8 DoubleRow (m0) plus
    three-column-group fp8 (m1), kv resident in SBUF, drained to bf16.
"""

import sys

sys.path.insert(0, "/opt/trn_rl_repo")

import numpy as np
import ml_dtypes

import concourse.bass as bass
import concourse.mybir as mybir

DT = mybir.dt
AF = mybir.ActivationFunctionType
PM = mybir.MatmulPerfMode

# problem geometry (hardcoded from the reference module)
KC, KH, KW = 128, 10, 4
H, W = 100, 64
NH, NW = H // KH, W // KW          # 10, 16
PQ = NH * NW                       # 160 q patches
D = KC * KH * KW                   # 5120
HK, WK = H - KH + 1, W - KW + 1    # 91, 61
NCORES = 8
HPC = 12                           # kv h-rows per core (8*12 = 96 >= 91)
PKC = HPC * W                      # 768 flat positions per core
ZROWS = 2 * HPC                    # 24 z rows staged per core
ZLEN = ZROWS * W                   # 1536
SCALE = 1.0 / D
LN64 = float(np.log(64.0))
F8 = ml_dtypes.float8_e4m3fn

_CACHE = {}


def _build_nc():
    nc = bass.Bass()
    zz_d = nc.declare_dram_parameter("zz", [KC, 2, ZLEN], DT.float8e4, isOutput=False)
    qm1_d = nc.declare_dram_parameter("qm1", [KC, KW, KH, 32], DT.float8e4, isOutput=False)
    idc_d = nc.declare_dram_parameter("idc", [128, 129], DT.float32, isOutput=False)
    qm0_d = nc.declare_dram_parameter("qm0", [KC, KW, KH, 128], DT.float8e4, isOutput=False)
    kv_d = nc.declare_dram_parameter("kv8", [128, 6, D], DT.float8e4, isOutput=False)
    ohi_d = nc.declare_dram_parameter("ohi", [128, D], DT.bfloat16, isOutput=True)
    olo_d = nc.declare_dram_parameter("olo", [96, 4, 512], DT.bfloat16, isOutput=True)
    den_d = nc.declare_dram_parameter("den", [224, 1], DT.float32, isOutput=True)

    from contextlib import ExitStack

    ctx = ExitStack()
    with ctx:
        zz_sb = ctx.enter_context(nc.sbuf_tensor([KC, 2, ZLEN], DT.float8e4))
        qm1_sb = ctx.enter_context(nc.sbuf_tensor([KC, KW, KH, 32], DT.float8e4))
        qm0_sb = ctx.enter_context(nc.sbuf_tensor([KC, KW, KH, 128], DT.float8e4))
        idc = ctx.enter_context(nc.sbuf_tensor([128, 129], DT.float32))
        kv_sb = ctx.enter_context(nc.sbuf_tensor([128, 6, D], DT.float8e4))
        e_hi = ctx.enter_context(nc.sbuf_tensor([128, PKC], DT.float32))
        e_lo = ctx.enter_context(nc.sbuf_tensor([96, 256], DT.float32))
        fT = ctx.enter_context(nc.sbuf_tensor([128, 6, PQ], DT.float8e4))
        o_hi = ctx.enter_context(nc.sbuf_tensor([128, D], DT.bfloat16))
        o_lo = ctx.enter_context(nc.sbuf_tensor([96, 4, 512], DT.bfloat16))
        dh_sb = ctx.enter_context(nc.sbuf_tensor([128, 1], DT.float32))
        dl_sb = ctx.enter_context(nc.sbuf_tensor([96, 1], DT.float32))
        scr = ctx.enter_context(nc.sbuf_tensor([128, 8], DT.float32))
        wz = ctx.enter_context(nc.sbuf_tensor([128, 512], DT.float8e4))

        ps_a = ctx.enter_context(nc.psum_tensor("ps_a", [128, 512], DT.float32))
        ps_b = ctx.enter_context(nc.psum_tensor("ps_b", [128, 512], DT.float32))
        ps_m = ctx.enter_context(nc.psum_tensor("ps_m", [128, 512], DT.float32))
        ps_w = ctx.enter_context(nc.psum_tensor("ps_w", [128, 512], DT.float32))
        ps_t = [
            ctx.enter_context(nc.psum_tensor(f"ps_t{i}", [128, 512], DT.float32))
            for i in range(4)
        ]

        s_z = ctx.enter_context(nc.semaphore("s_z"))
        s_q1 = ctx.enter_context(nc.semaphore("s_q1"))
        s_q0 = ctx.enter_context(nc.semaphore("s_q0"))
        s_i = ctx.enter_context(nc.semaphore("s_i"))
        s_kv = [ctx.enter_context(nc.semaphore(f"s_kv{i}")) for i in range(3)]
        s_p = ctx.enter_context(nc.semaphore("s_p"))
        s_a = ctx.enter_context(nc.semaphore("s_a"))
        s_v = ctx.enter_context(nc.semaphore("s_v"))
        s_o = ctx.enter_context(nc.semaphore("s_o"))
        s_g = ctx.enter_context(nc.semaphore("s_g"))

        # p1 m1 col-groups: gA->ps_m[0:32], gB->ps_t0[32:64], gC->ps_t1[64:96]
        M1B = [ps_m, ps_t[0], ps_t[1]]
        # TR1 transposes (m1 e_lo chunks 0..5) alternate banks t2/t3
        TR1_BANK = [ps_t[2], ps_t[3], ps_t[2], ps_t[3], ps_t[2], ps_t[3]]
        TR1_SA = [1, 1, 2, 2, 3, 3]       # e_lo chunk needs its group's exp
        TR1_SV = [0, 0, 2, 3, 4, 5]       # bank drain (DVE fT copy) for reuse
        # TR0 emission order: e_hi chunks [4,5,0,1,2,3] on banks [t0,t1,t2,t3,t0,t1]
        TR0_CHUNK = [4, 5, 0, 1, 2, 3]
        TR0_BANK = [ps_t[0], ps_t[1], ps_t[2], ps_t[3], ps_t[0], ps_t[1]]
        TR0_SA = [4, 4, 5, 5, 5, 5]       # chunks 4,5 <- exp B; 0..3 <- exp A
        TR0_SV = [0, 0, 6, 7, 8, 9]
        # phase-2 m1 triples (n-tiles 3r..3r+2) on banks (w, b, m)
        RB = [ps_w, ps_b, ps_m]
        TRIP_SA = [4, 7, 9, 11]
        TRIP_SV = [7, 10, 16, 0]
        TRIP_KV = [[0], [1], [1, 2], [2]]

        with nc.Block() as block:

            @block.sync
            def _(sync):
                sync.dma_start(zz_sb[:, :, :], zz_d[:]).then_inc(s_z, 16)
                sync.dma_start(qm1_sb[:, :, :, :], qm1_d[:]).then_inc(s_q1, 16)
                sync.dma_start(idc[:, :], idc_d[:]).then_inc(s_i, 16)
                sync.dma_start(qm0_sb[:, :, :, :], qm0_d[:]).then_inc(s_q0, 16)
                sync.dma_start(kv_sb[:, :, 0:1536], kv_d[:, :, 0:1536]).then_inc(
                    s_kv[0], 16
                )
                sync.dma_start(kv_sb[:, :, 1536:3584], kv_d[:, :, 1536:3584]).then_inc(
                    s_kv[1], 16
                )
                sync.dma_start(kv_sb[:, :, 3584:5120], kv_d[:, :, 3584:5120]).then_inc(
                    s_kv[2], 16
                )
                sync.wait_ge(s_v, 1)
                sync.dma_start(den_d[128:224, :], dl_sb[:]).then_inc(s_o, 16)
                sync.wait_ge(s_a, 7)
                sync.wait_ge(s_v, 10)
                sync.dma_start(olo_d[:, 0, :], o_lo[:, 0, :]).then_inc(s_o, 16)
                sync.wait_ge(s_v, 15)
                sync.dma_start(den_d[0:128, :], dh_sb[:]).then_inc(s_o, 16)
                sync.wait_ge(s_a, 9)
                sync.wait_ge(s_v, 16)
                sync.dma_start(olo_d[:, 1, :], o_lo[:, 1, :]).then_inc(s_o, 16)
                sync.wait_ge(s_a, 10)
                sync.wait_ge(s_v, 17)
                sync.dma_start(ohi_d[:, 0:1024], o_hi[:, 0:1024]).then_inc(s_o, 16)
                sync.wait_ge(s_a, 12)
                sync.wait_ge(s_v, 18)
                sync.dma_start(olo_d[:, 2, :], o_lo[:, 2, :]).then_inc(s_o, 16)
                sync.wait_ge(s_a, 13)
                sync.wait_ge(s_v, 19)
                sync.dma_start(ohi_d[:, 1024:2048], o_hi[:, 1024:2048]).then_inc(
                    s_o, 16
                )
                sync.wait_ge(s_a, 14)
                sync.wait_ge(s_v, 20)
                sync.dma_start(ohi_d[:, 2048:3072], o_hi[:, 2048:3072]).then_inc(
                    s_o, 16
                )
                sync.wait_ge(s_a, 15)
                sync.wait_ge(s_v, 21)
                sync.dma_start(ohi_d[:, 3072:4096], o_hi[:, 3072:4096]).then_inc(
                    s_o, 16
                )
                sync.wait_ge(s_a, 16)
                sync.wait_ge(s_v, 22)
                sync.dma_start(ohi_d[:, 4096:5120], o_hi[:, 4096:5120]).then_inc(
                    s_o, 16
                )
                sync.wait_ge(s_a, 17)
                sync.dma_start(olo_d[0:32, 3, :], o_lo[0:32, 3, :]).then_inc(s_o, 16)
                sync.wait_ge(s_o, 176)

            @block.tensor
            def _(pe):
                # HAM warmup on the DVE-memset tile until the z/q DMAs land
                pe.wait_ge(s_g, 1)
                for w in range(9):
                    nc.tensor.matmul(
                        ps_w[0:128, 0:512],
                        wz[:, 0:128],
                        wz[:, 0:512],
                        start=(w == 0),
                        stop=(w == 8),
                    )
                pe.wait_ge(s_z, 16)
                pe.wait_ge(s_q1, 16)
                # phase 1 m1 (q rows 128..159): 3 concurrent 32-col groups
                mfin = [None, None, None]
                for i in range(KH):
                    for j in range(KW):
                        st = i == 0 and j == 0
                        sp = i == KH - 1 and j == KW - 1
                        off = i * W + j
                        for g in range(3):
                            mfin[g] = nc.tensor.matmul(
                                M1B[g][32 * g : 32 * g + 32, 0:256],
                                qm1_sb[:, j, i, :],
                                zz_sb[:, 0, off + 256 * g : off + 256 * g + 256],
                                start=st,
                                stop=sp,
                            )
                for g in range(3):
                    mfin[g].then_inc(s_p, 1)  # s_p = 1, 2, 3
                pe.wait_ge(s_q0, 16)
                # phase 1 m0 chain B (pos 512:768), DoubleRow (i, i+1) pairs
                for j in range(KW):
                    for ip in range(5):
                        st = j == 0 and ip == 0
                        sp = j == KW - 1 and ip == 4
                        off = (2 * ip) * W + j
                        mm = nc.tensor.matmul(
                            ps_b[0:128, 0:256],
                            qm0_sb[:, j, 2 * ip : 2 * ip + 2, :],
                            zz_sb[:, :, off + 512 : off + 768],
                            start=st,
                            stop=sp,
                            perf_mode=PM.DoubleRow,
                        )
                mm.then_inc(s_p, 1)  # s_p = 4
                # TR1: transpose e_lo (m1) chunks -> banks t2/t3
                pe.wait_ge(s_i, 16)
                for k in range(6):
                    g = k // 2
                    col = (k % 2) * 128
                    pe.wait_ge(s_a, TR1_SA[k])
                    if TR1_SV[k]:
                        pe.wait_ge(s_v, TR1_SV[k])
                    nc.tensor.matmul(
                        TR1_BANK[k][0:128, 0:32],
                        e_lo[32 * g : 32 * g + 32, col : col + 128],
                        idc[32 * g : 32 * g + 32, 32 * g : 32 * g + 32],
                        is_transpose=True,
                        start=True,
                        stop=True,
                    ).then_inc(s_p, 1)  # s_p = 5..10
                # phase 1 m0 chain A (pos 0:512)
                for j in range(KW):
                    for ip in range(5):
                        st = j == 0 and ip == 0
                        sp = j == KW - 1 and ip == 4
                        off = (2 * ip) * W + j
                        mm = nc.tensor.matmul(
                            ps_a[0:128, 0:512],
                            qm0_sb[:, j, 2 * ip : 2 * ip + 2, :],
                            zz_sb[:, :, off : off + 512],
                            start=st,
                            stop=sp,
                            perf_mode=PM.DoubleRow,
                        )
                mm.then_inc(s_p, 1)  # s_p = 11

                def p2triple(r):
                    pe.wait_ge(s_a, TRIP_SA[r])
                    if TRIP_SV[r]:
                        pe.wait_ge(s_v, TRIP_SV[r])
                    for pc in TRIP_KV[r]:
                        pe.wait_ge(s_kv[pc], 16)
                    ng = 3 if r < 3 else 1
                    mf = [None] * ng
                    for t6 in range(6):
                        st, sp = t6 == 0, t6 == 5
                        for g in range(ng):
                            mf[g] = nc.tensor.matmul(
                                RB[g][32 * g : 32 * g + 32, 0:512],
                                fT[:, t6, 128:160],
                                kv_sb[:, t6, (3 * r + g) * 512 : (3 * r + g + 1) * 512],
                                start=st,
                                stop=sp,
                            )
                    for g in range(ng):
                        mf[g].then_inc(s_p, 1)

                # r0 runs in the exp-A latency shadow right after chain A
                p2triple(0)  # s_p = 12, 13, 14
                # TR0: transpose e_hi chunks, exp-B-dependent chunks first
                for k in range(6):
                    c = TR0_CHUNK[k]
                    pe.wait_ge(s_a, TR0_SA[k])
                    if TR0_SV[k]:
                        pe.wait_ge(s_v, TR0_SV[k])
                    nc.tensor.matmul(
                        TR0_BANK[k][0:128, 0:128],
                        e_hi[:, c * 128 : (c + 1) * 128],
                        idc[0:128, 0:128],
                        is_transpose=True,
                        start=True,
                        stop=True,
                    ).then_inc(s_p, 1)  # s_p = 15..20

                def p2pair(k, banks, sa, sv, kvs):
                    if sa:
                        pe.wait_ge(s_a, sa)
                    pe.wait_ge(s_v, sv)
                    for pc in kvs:
                        pe.wait_ge(s_kv[pc], 16)
                    bA, bB = ps_t[banks[0]], ps_t[banks[1]]
                    for tp in range(3):
                        st, sp = tp == 0, tp == 2
                        mA = nc.tensor.matmul(
                            bA[0:128, 0:512],
                            fT[:, 2 * tp : 2 * tp + 2, 0:128],
                            kv_sb[
                                :, 2 * tp : 2 * tp + 2, (2 * k) * 512 : (2 * k + 1) * 512
                            ],
                            start=st,
                            stop=sp,
                            perf_mode=PM.DoubleRow,
                        )
                        mB = nc.tensor.matmul(
                            bB[0:128, 0:512],
                            fT[:, 2 * tp : 2 * tp + 2, 0:128],
                            kv_sb[
                                :,
                                2 * tp : 2 * tp + 2,
                                (2 * k + 1) * 512 : (2 * k + 2) * 512,
                            ],
                            start=st,
                            stop=sp,
                            perf_mode=PM.DoubleRow,
                        )
                    mA.then_inc(s_p, 1)
                    mB.then_inc(s_p, 1)

                p2triple(1)                       # s_p = 21, 22, 23
                p2pair(0, (2, 3), 0, 14, [0])     # s_p = 24, 25
                p2triple(2)                       # s_p = 26, 27, 28
                p2pair(1, (0, 1), 0, 14, [0, 1])  # s_p = 29, 30
                p2pair(2, (2, 3), 10, 17, [1])    # s_p = 31, 32
                p2pair(3, (0, 1), 13, 19, [1, 2])  # s_p = 33, 34
                p2pair(4, (2, 3), 14, 20, [2])    # s_p = 35, 36
                p2triple(3)                       # s_p = 37

            @block.scalar
            def _(act):
                # warm the exp table set early (reads the memset tile)
                act.wait_ge(s_g, 1)
                nc.scalar.activation(
                    scr[:, :], wz[:, 0:8], AF.Exp, bias=0.0, scale=1.0
                )
                act.wait_ge(s_i, 16)
                # e64 = 64 * exp(s * SCALE); bias AP holds ln(64)
                for g in range(3):
                    act.wait_ge(s_p, 1 + g)
                    nc.scalar.activation(
                        e_lo[32 * g : 32 * g + 32, 0:256],
                        M1B[g][32 * g : 32 * g + 32, 0:256],
                        AF.Exp,
                        bias=idc[32 * g : 32 * g + 32, 128:129],
                        scale=SCALE,
                    ).then_inc(s_a, 1)  # 1, 2, 3
                act.wait_ge(s_p, 4)
                nc.scalar.activation(
                    e_hi[:, 512:768], ps_b[0:128, 0:256], AF.Exp,
                    bias=idc[:, 128:129], scale=SCALE,
                ).then_inc(s_a, 1)  # 4
                act.wait_ge(s_p, 11)
                nc.scalar.activation(
                    e_hi[:, 0:512], ps_a[0:128, 0:512], AF.Exp,
                    bias=idc[:, 128:129], scale=SCALE,
                ).then_inc(s_a, 1)  # 5

                def m1drain(gi, r, spv):
                    act.wait_ge(s_p, spv)
                    nc.scalar.activation(
                        o_lo[32 * gi : 32 * gi + 32, r, :],
                        RB[gi][32 * gi : 32 * gi + 32, 0:512],
                        AF.Copy,
                    ).then_inc(s_a, 1)

                def ntdrain_a(g, spv, bank):
                    act.wait_ge(s_p, spv)
                    nc.scalar.activation(
                        o_hi[:, g * 512 : (g + 1) * 512],
                        bank[0:128, 0:512],
                        AF.Copy,
                    ).then_inc(s_a, 1)

                m1drain(0, 0, 12)            # 6
                m1drain(1, 0, 13)            # 7
                m1drain(0, 1, 21)            # 8
                m1drain(1, 1, 22)            # 9
                ntdrain_a(1, 25, ps_t[3])    # 10
                m1drain(0, 2, 26)            # 11
                m1drain(1, 2, 27)            # 12
                ntdrain_a(3, 30, ps_t[1])    # 13
                ntdrain_a(5, 32, ps_t[3])    # 14
                ntdrain_a(7, 34, ps_t[1])    # 15
                ntdrain_a(9, 36, ps_t[3])    # 16
                m1drain(0, 3, 37)            # 17

            @block.vector
            def _(dve):
                nc.vector.memset(wz[:, :], 0.0).then_inc(s_g, 1)
                dve.wait_ge(s_a, 3)
                nc.vector.reduce_sum(
                    dl_sb[:], e_lo[:, :], axis=mybir.AxisListType.X
                ).then_inc(s_v, 1)  # 1
                # fT copies: f64 = e64T - 64, cast to fp8
                for k in range(6):
                    dve.wait_ge(s_p, 5 + k)
                    nc.vector.tensor_scalar_sub(
                        fT[:, k, 128:160], TR1_BANK[k][0:128, 0:32], 64.0
                    ).then_inc(s_v, 1)  # 2..7
                def gc_drain(r, sp_val):
                    dve.wait_ge(s_p, sp_val)
                    nc.vector.tensor_copy(
                        o_lo[64:96, r, :], ps_m[64:96, 0:512]
                    ).then_inc(s_v, 1)

                def nt_drain(g, sp_val, bank):
                    dve.wait_ge(s_p, sp_val)
                    nc.vector.tensor_copy(
                        o_hi[:, g * 512 : (g + 1) * 512], bank[0:128, 0:512]
                    ).then_inc(s_v, 1)

                for k in range(2):
                    dve.wait_ge(s_p, 15 + k)
                    nc.vector.tensor_scalar_sub(
                        fT[:, TR0_CHUNK[k], 0:128], TR0_BANK[k][0:128, 0:128], 64.0
                    ).then_inc(s_v, 1)  # 8, 9
                gc_drain(0, 14)            # 10
                for k in range(2, 6):
                    dve.wait_ge(s_p, 15 + k)
                    nc.vector.tensor_scalar_sub(
                        fT[:, TR0_CHUNK[k], 0:128], TR0_BANK[k][0:128, 0:128], 64.0
                    ).then_inc(s_v, 1)  # 11..14
                dve.wait_ge(s_a, 5)
                nc.vector.reduce_sum(
                    dh_sb[:], e_hi[:, :], axis=mybir.AxisListType.X
                ).then_inc(s_v, 1)  # 15
                gc_drain(1, 23)            # 16
                nt_drain(0, 24, ps_t[2])   # 17
                gc_drain(2, 28)            # 18
                nt_drain(2, 29, ps_t[0])   # 19
                nt_drain(4, 31, ps_t[2])   # 20
                nt_drain(6, 33, ps_t[0])   # 21
                nt_drain(8, 35, ps_t[2])   # 22

    return nc


def _host_prep(z1_hat, z2):
    z1 = np.asarray(z1_hat, dtype=np.float32)[0]  # [128, 100, 64]
    z2a = np.asarray(z2, dtype=np.float32)[0]

    # q patches [160, 5120]; device layout q8 [128, j, i, 160]
    q = z1.reshape(KC, NH, KH, NW, KW).transpose(1, 3, 0, 2, 4).reshape(PQ, D)
    q4 = q.reshape(PQ, KC, KH, KW)
    q8 = q4.transpose(1, 3, 2, 0).astype(F8)  # [c, j, i, p]
    qm0 = np.ascontiguousarray(q8[:, :, :, 0:128])
    qm1 = np.ascontiguousarray(q8[:, :, :, 128:160])

    # padded z2: rows 100..111 zero
    z_pad = np.zeros((KC, 112, W), dtype=np.float32)
    z_pad[:, :H] = z2a
    z8_pad = z_pad.astype(F8)

    # sliding kv patches from padded z2 (original fp32 values, cast per-row)
    sw = np.lib.stride_tricks.sliding_window_view(z_pad, (KH, KW), axis=(1, 2))

    q64 = q.astype(np.float64)
    ij_off = (np.arange(KH)[:, None] * W + np.arange(KW)[None, :]).reshape(-1)  # [40]

    idc = np.zeros((128, 129), dtype=np.float32)
    idc[:, 0:128] = np.eye(128, dtype=np.float32)
    idc[:, 128] = LN64

    in_maps = []
    corrs = []
    for core in range(NCORES):
        h0 = HPC * core
        slab8 = z8_pad[:, h0 : h0 + ZROWS, :].reshape(KC, ZLEN)
        zz = np.zeros((KC, 2, ZLEN), dtype=F8)
        zz[:, 0, :] = slab8
        zz[:, 1, 0 : ZLEN - W] = slab8[:, W:]
        # kv rows indexed by flat position p = h_local*64 + w
        kvp = np.zeros((PKC, D), dtype=F8)
        hh = np.arange(PKC) // W
        ww = np.arange(PKC) % W
        real = (ww < WK) & (h0 + hh < HK)
        ridx = np.nonzero(real)[0]
        kvp[ridx] = (
            sw[:, h0 + hh[ridx], ww[ridx]].transpose(1, 0, 2, 3).reshape(-1, D)
        ).astype(F8)
        kv8 = np.ascontiguousarray(kvp.reshape(6, 128, D).transpose(1, 0, 2))
        in_maps.append(
            {"zz": zz, "qm0": qm0, "qm1": qm1, "kv8": kv8, "idc": idc}
        )
        # denominator correction: computed-but-invalid stream positions,
        # recomputed exactly (fp64) from the original values.
        bad = np.nonzero(~real)[0]
        zf = z_pad[:, h0 : h0 + ZROWS, :].reshape(KC, ZLEN).astype(np.float64)
        win = zf[:, bad[:, None] + ij_off[None, :]]  # [128, nb, 40]
        patches = win.transpose(1, 0, 2).reshape(len(bad), D)
        s_bad = q64 @ patches.T  # [160, nb]
        corrs.append(np.exp(s_bad * SCALE).sum(axis=1))

    corr = np.sum(corrs, axis=0)
    swr = sw[:, :HK, :WK]
    colsum = swr.astype(np.float64).sum(axis=(1, 2)).reshape(D)  # [5120]
    return in_maps, corr, colsum


def kernel(z1_hat, z2):
    from concourse.bass_utils import run_bass_kernel_spmd

    in_maps, corr, colsum = _host_prep(z1_hat, z2)
    if "nc" not in _CACHE:
        _CACHE["nc"] = _build_nc()
    nc = _CACHE["nc"]
    res = run_bass_kernel_spmd(nc, in_maps, list(range(NCORES)))
    num = np.broadcast_to(colsum, (PQ, D)).astype(np.float64).copy()
    den = -corr
    for r in res.results:
        ohi = r["ohi"].astype(np.float64)  # [128, 5120] = 64 * partial m0
        olo = r["olo"].astype(np.float64).reshape(96, 4, 512)
        part = np.zeros((PQ, D))
        part[0:128] = ohi
        for nt in range(10):
            rr, g = nt // 3, nt % 3
            part[128:160, nt * 512 : (nt + 1) * 512] = olo[32 * g : 32 * g + 32, rr]
        num += part / 64.0
        dv = r["den"].astype(np.float64)[:, 0] / 64.0
        den = den + np.concatenate(
            [dv[0:128], dv[128:160] + dv[160:192] + dv[192:224]]
        )
    out = (num / den[:, None]).astype(np.float32)
    # fold patches back: [160, 5120] -> [1, 128, 100, 64]
    out = out.reshape(NH, NW, KC, KH, KW).transpose(2, 0, 3, 1, 4)
    return np.ascontiguousarray(out.reshape(1, KC, H, W))


# revision 17
# speedup vs baseline: 1.5081x; 1.0172x over previous
"""Trainium2 Bass kernel for BottleneckAttention (patch attention), fp8 edition.

q patches [160, 5120] from z1_hat (non-overlapping 10x4 unfold),
kv patches [5551, 5120] from z2 (overlapping unfold, Hk=91 x Wk=61),
scores = q @ kv.T / 5120, softmax over kv patches, out = attn @ kv,
folded back to [1, 128, 100, 64].

Sharding: contiguous blocks of 12 kv h-rows per core (8 x 12 = 96 >= 91).
Each core owns the 768 flat positions p = h_local*64 + w (w in [0,64));
positions with w >= 61 or h >= 91 are invalid -- their kv rows are zeroed
so they never touch the numerator, and the host subtracts their exactly
recomputed exp contribution from the denominator. Every core computes all
160 q rows; the host combines with an all-gather softmax.

Per-core kernel (raw Bass, explicit semaphores), fp8e4m3 everywhere on
the PE with DoubleRow (K=256) perf mode for the M=128 matmul blocks:
  phase 1: scores as implicit convolution against the SBUF-resident
    z2 slab (zz holds the slab plus a 64-shifted copy so (i, i+1) kernel
    row pairs form clean [128, 2, N] DoubleRow moving operands).
    q rows 0..127 run M=128 DoubleRow; rows 128..159 run as three
    concurrent 32-wide PE column groups (no DoubleRow -- col tiling and
    DoubleRow are mutually exclusive).
  exp on ScalarE with scale=1/5120 and bias=ln(64): e64 = 64*exp(s).
  row-sum denominator (64x) on VectorE; host divides by 64.
  PE transposes of e64 chunks; the ACT psum->sbuf copy applies bias=-64
  so the fp8 fT stores f64 = 64*(e-1) (centered softmax, scaled into
  fp8e4m3's normal range; the host adds the exact sum-of-kv-columns term
  and divides by 64).
  phase 2: partial_out = f64T.T @ kv_shard in fp8 DoubleRow (m0) plus
    three-column-group fp8 (m1), kv resident in SBUF, drained to bf16.
"""

import sys

sys.path.insert(0, "/opt/trn_rl_repo")

import numpy as np
import ml_dtypes

import concourse.bass as bass
import concourse.mybir as mybir

DT = mybir.dt
AF = mybir.ActivationFunctionType
PM = mybir.MatmulPerfMode

# problem geometry (hardcoded from the reference module)
KC, KH, KW = 128, 10, 4
H, W = 100, 64
NH, NW = H // KH, W // KW          # 10, 16
PQ = NH * NW                       # 160 q patches
D = KC * KH * KW                   # 5120
HK, WK = H - KH + 1, W - KW + 1    # 91, 61
NCORES = 8
HPC = 12                           # kv h-rows per core (8*12 = 96 >= 91)
PKC = HPC * W                      # 768 flat positions per core
ZROWS = 2 * HPC                    # 24 z rows staged per core
ZLEN = ZROWS * W                   # 1536
SCALE = 1.0 / D
LN64 = float(np.log(64.0))
F8 = ml_dtypes.float8_e4m3fn

_CACHE = {}


def _build_nc():
    nc = bass.Bass()
    zz_d = nc.declare_dram_parameter("zz", [KC, 2, ZLEN], DT.float8e4, isOutput=False)
    qm1_d = nc.declare_dram_parameter("qm1", [KC, KW, KH, 32], DT.float8e4, isOutput=False)
    idc_d = nc.declare_dram_parameter("idc", [128, 129], DT.float32, isOutput=False)
    qm0_d = nc.declare_dram_parameter("qm0", [KC, KW, KH, 128], DT.float8e4, isOutput=False)
    kv_d = nc.declare_dram_parameter("kv8", [128, 6, D], DT.float8e4, isOutput=False)
    ohi_d = nc.declare_dram_parameter("ohi", [128, D], DT.bfloat16, isOutput=True)
    olo_d = nc.declare_dram_parameter("olo", [96, 4, 512], DT.bfloat16, isOutput=True)
    den_d = nc.declare_dram_parameter("den", [224, 1], DT.float32, isOutput=True)

    from contextlib import ExitStack

    ctx = ExitStack()
    with ctx:
        zz_sb = ctx.enter_context(nc.sbuf_tensor([KC, 2, ZLEN], DT.float8e4))
        qm1_sb = ctx.enter_context(nc.sbuf_tensor([KC, KW, KH, 32], DT.float8e4))
        qm0_sb = ctx.enter_context(nc.sbuf_tensor([KC, KW, KH, 128], DT.float8e4))
        idc = ctx.enter_context(nc.sbuf_tensor([128, 129], DT.float32))
        kv_sb = ctx.enter_context(nc.sbuf_tensor([128, 6, D], DT.float8e4))
        e_hi = ctx.enter_context(nc.sbuf_tensor([128, PKC], DT.float32))
        e_lo = ctx.enter_context(nc.sbuf_tensor([96, 256], DT.float32))
        fT = ctx.enter_context(nc.sbuf_tensor([128, 6, PQ], DT.float8e4))
        o_hi = ctx.enter_context(nc.sbuf_tensor([128, D], DT.bfloat16))
        o_lo = ctx.enter_context(nc.sbuf_tensor([96, 4, 512], DT.bfloat16))
        dh_sb = ctx.enter_context(nc.sbuf_tensor([128, 1], DT.float32))
        dl_sb = ctx.enter_context(nc.sbuf_tensor([96, 1], DT.float32))
        scr = ctx.enter_context(nc.sbuf_tensor([128, 8], DT.float32))
        wz = ctx.enter_context(nc.sbuf_tensor([128, 512], DT.float8e4))

        ps_a = ctx.enter_context(nc.psum_tensor("ps_a", [128, 512], DT.float32))
        ps_b = ctx.enter_context(nc.psum_tensor("ps_b", [128, 512], DT.float32))
        ps_m = ctx.enter_context(nc.psum_tensor("ps_m", [128, 512], DT.float32))
        ps_w = ctx.enter_context(nc.psum_tensor("ps_w", [128, 512], DT.float32))
        ps_t = [
            ctx.enter_context(nc.psum_tensor(f"ps_t{i}", [128, 512], DT.float32))
            for i in range(4)
        ]

        s_z = ctx.enter_context(nc.semaphore("s_z"))
        s_z2 = ctx.enter_context(nc.semaphore("s_z2"))
        s_q1 = ctx.enter_context(nc.semaphore("s_q1"))
        s_q0 = ctx.enter_context(nc.semaphore("s_q0"))
        s_i = ctx.enter_context(nc.semaphore("s_i"))
        s_kv = [ctx.enter_context(nc.semaphore(f"s_kv{i}")) for i in range(3)]
        s_p = ctx.enter_context(nc.semaphore("s_p"))
        s_a = ctx.enter_context(nc.semaphore("s_a"))
        s_v = ctx.enter_context(nc.semaphore("s_v"))
        s_o = ctx.enter_context(nc.semaphore("s_o"))
        s_g = ctx.enter_context(nc.semaphore("s_g"))

        # p1 m1 col-groups: gA->ps_m[0:32], gB->ps_t0[32:64], gC->ps_t1[64:96]
        M1B = [ps_m, ps_t[0], ps_t[1]]
        # TR1 transposes (m1 e_lo chunks 0..5) alternate banks t2/t3
        TR1_BANK = [ps_t[2], ps_t[3], ps_t[2], ps_t[3], ps_t[2], ps_t[3]]
        TR1_SA = [1, 1, 2, 2, 3, 3]       # e_lo chunk needs its group's exp
        TR1_SV = [0, 0, 2, 3, 4, 5]       # bank drain (DVE fT copy) for reuse
        # TR0 emission order: e_hi chunks [4,5,0,1,2,3] on banks [t0,t1,t2,t3,t0,t1]
        TR0_CHUNK = [4, 5, 0, 1, 2, 3]
        TR0_BANK = [ps_t[0], ps_t[1], ps_t[2], ps_t[3], ps_t[0], ps_t[1]]
        TR0_SA = [4, 4, 5, 5, 5, 5]       # chunks 4,5 <- exp B; 0..3 <- exp A
        TR0_SV = [0, 0, 6, 7, 8, 9]
        # phase-2 m1 triples (n-tiles 3r..3r+2) on banks (w, b, m)
        RB = [ps_w, ps_b, ps_m]
        TRIP_SA = [4, 7, 9, 11]
        TRIP_SV = [7, 10, 16, 0]
        TRIP_KV = [[0], [1], [1, 2], [2]]

        with nc.Block() as block:

            @block.sync
            def _(sync):
                sync.dma_start(zz_sb[:, 0, :], zz_d[:, 0, :]).then_inc(s_z, 16)
                sync.dma_start(qm1_sb[:, :, :, :], qm1_d[:]).then_inc(s_q1, 16)
                sync.dma_start(idc[:, :], idc_d[:]).then_inc(s_i, 16)
                sync.dma_start(zz_sb[:, 1, :], zz_d[:, 1, :]).then_inc(s_z2, 16)
                sync.dma_start(qm0_sb[:, :, :, :], qm0_d[:]).then_inc(s_q0, 16)
                sync.dma_start(kv_sb[:, :, 0:1536], kv_d[:, :, 0:1536]).then_inc(
                    s_kv[0], 16
                )
                sync.dma_start(kv_sb[:, :, 1536:3584], kv_d[:, :, 1536:3584]).then_inc(
                    s_kv[1], 16
                )
                sync.dma_start(kv_sb[:, :, 3584:5120], kv_d[:, :, 3584:5120]).then_inc(
                    s_kv[2], 16
                )
                sync.wait_ge(s_v, 1)
                sync.dma_start(den_d[128:224, :], dl_sb[:]).then_inc(s_o, 16)
                sync.wait_ge(s_a, 7)
                sync.wait_ge(s_v, 10)
                sync.dma_start(olo_d[:, 0, :], o_lo[:, 0, :]).then_inc(s_o, 16)
                sync.wait_ge(s_v, 15)
                sync.dma_start(den_d[0:128, :], dh_sb[:]).then_inc(s_o, 16)
                sync.wait_ge(s_a, 9)
                sync.wait_ge(s_v, 16)
                sync.dma_start(olo_d[:, 1, :], o_lo[:, 1, :]).then_inc(s_o, 16)
                sync.wait_ge(s_a, 10)
                sync.wait_ge(s_v, 17)
                sync.dma_start(ohi_d[:, 0:1024], o_hi[:, 0:1024]).then_inc(s_o, 16)
                sync.wait_ge(s_a, 12)
                sync.wait_ge(s_v, 18)
                sync.dma_start(olo_d[:, 2, :], o_lo[:, 2, :]).then_inc(s_o, 16)
                sync.wait_ge(s_a, 13)
                sync.wait_ge(s_v, 19)
                sync.dma_start(ohi_d[:, 1024:2048], o_hi[:, 1024:2048]).then_inc(
                    s_o, 16
                )
                sync.wait_ge(s_a, 14)
                sync.wait_ge(s_v, 20)
                sync.dma_start(ohi_d[:, 2048:3072], o_hi[:, 2048:3072]).then_inc(
                    s_o, 16
                )
                sync.wait_ge(s_a, 15)
                sync.wait_ge(s_v, 21)
                sync.dma_start(ohi_d[:, 3072:4096], o_hi[:, 3072:4096]).then_inc(
                    s_o, 16
                )
                sync.wait_ge(s_a, 16)
                sync.dma_start(olo_d[0:32, 3, :], o_lo[0:32, 3, :]).then_inc(s_o, 16)
                sync.wait_ge(s_a, 17)
                sync.wait_ge(s_v, 22)
                sync.dma_start(ohi_d[:, 4096:5120], o_hi[:, 4096:5120]).then_inc(
                    s_o, 16
                )
                sync.wait_ge(s_o, 176)

            @block.tensor
            def _(pe):
                # HAM warmup on the DVE-memset tile until the z/q DMAs land
                pe.wait_ge(s_g, 1)
                for w in range(6):
                    nc.tensor.matmul(
                        ps_w[0:128, 0:512],
                        wz[:, 0:128],
                        wz[:, 0:512],
                        start=(w == 0),
                        stop=(w == 5),
                    )
                pe.wait_ge(s_z, 16)
                pe.wait_ge(s_q1, 16)
                # phase 1 m1 (q rows 128..159): 3 concurrent 32-col groups
                mfin = [None, None, None]
                for i in range(KH):
                    for j in range(KW):
                        st = i == 0 and j == 0
                        sp = i == KH - 1 and j == KW - 1
                        off = i * W + j
                        for g in range(3):
                            mfin[g] = nc.tensor.matmul(
                                M1B[g][32 * g : 32 * g + 32, 0:256],
                                qm1_sb[:, j, i, :],
                                zz_sb[:, 0, off + 256 * g : off + 256 * g + 256],
                                start=st,
                                stop=sp,
                            )
                for g in range(3):
                    mfin[g].then_inc(s_p, 1)  # s_p = 1, 2, 3
                pe.wait_ge(s_q0, 16)
                pe.wait_ge(s_z2, 16)
                # phase 1 m0 chain B (pos 512:768), DoubleRow (i, i+1) pairs
                for j in range(KW):
                    for ip in range(5):
                        st = j == 0 and ip == 0
                        sp = j == KW - 1 and ip == 4
                        off = (2 * ip) * W + j
                        mm = nc.tensor.matmul(
                            ps_b[0:128, 0:256],
                            qm0_sb[:, j, 2 * ip : 2 * ip + 2, :],
                            zz_sb[:, :, off + 512 : off + 768],
                            start=st,
                            stop=sp,
                            perf_mode=PM.DoubleRow,
                        )
                mm.then_inc(s_p, 1)  # s_p = 4
                # TR1: transpose e_lo (m1) chunks -> banks t2/t3
                pe.wait_ge(s_i, 16)
                for k in range(6):
                    g = k // 2
                    col = (k % 2) * 128
                    pe.wait_ge(s_a, TR1_SA[k])
                    if TR1_SV[k]:
                        pe.wait_ge(s_v, TR1_SV[k])
                    nc.tensor.matmul(
                        TR1_BANK[k][0:128, 0:32],
                        e_lo[32 * g : 32 * g + 32, col : col + 128],
                        idc[32 * g : 32 * g + 32, 32 * g : 32 * g + 32],
                        is_transpose=True,
                        start=True,
                        stop=True,
                    ).then_inc(s_p, 1)  # s_p = 5..10
                # phase 1 m0 chain A (pos 0:512)
                for j in range(KW):
                    for ip in range(5):
                        st = j == 0 and ip == 0
                        sp = j == KW - 1 and ip == 4
                        off = (2 * ip) * W + j
                        mm = nc.tensor.matmul(
                            ps_a[0:128, 0:512],
                            qm0_sb[:, j, 2 * ip : 2 * ip + 2, :],
                            zz_sb[:, :, off : off + 512],
                            start=st,
                            stop=sp,
                            perf_mode=PM.DoubleRow,
                        )
                mm.then_inc(s_p, 1)  # s_p = 11

                def p2triple(r):
                    pe.wait_ge(s_a, TRIP_SA[r])
                    if TRIP_SV[r]:
                        pe.wait_ge(s_v, TRIP_SV[r])
                    for pc in TRIP_KV[r]:
                        pe.wait_ge(s_kv[pc], 16)
                    ng = 3 if r < 3 else 1
                    mf = [None] * ng
                    for t6 in range(6):
                        st, sp = t6 == 0, t6 == 5
                        for g in range(ng):
                            mf[g] = nc.tensor.matmul(
                                RB[g][32 * g : 32 * g + 32, 0:512],
                                fT[:, t6, 128:160],
                                kv_sb[:, t6, (3 * r + g) * 512 : (3 * r + g + 1) * 512],
                                start=st,
                                stop=sp,
                            )
                    for g in range(ng):
                        mf[g].then_inc(s_p, 1)

                # r0 runs in the exp-A latency shadow right after chain A
                p2triple(0)  # s_p = 12, 13, 14
                # TR0: transpose e_hi chunks, exp-B-dependent chunks first
                for k in range(6):
                    c = TR0_CHUNK[k]
                    pe.wait_ge(s_a, TR0_SA[k])
                    if TR0_SV[k]:
                        pe.wait_ge(s_v, TR0_SV[k])
                    nc.tensor.matmul(
                        TR0_BANK[k][0:128, 0:128],
                        e_hi[:, c * 128 : (c + 1) * 128],
                        idc[0:128, 0:128],
                        is_transpose=True,
                        start=True,
                        stop=True,
                    ).then_inc(s_p, 1)  # s_p = 15..20

                def p2pair(k, banks, sa, sv, kvs):
                    if sa:
                        pe.wait_ge(s_a, sa)
                    pe.wait_ge(s_v, sv)
                    for pc in kvs:
                        pe.wait_ge(s_kv[pc], 16)
                    bA, bB = ps_t[banks[0]], ps_t[banks[1]]
                    for tp in range(3):
                        st, sp = tp == 0, tp == 2
                        mA = nc.tensor.matmul(
                            bA[0:128, 0:512],
                            fT[:, 2 * tp : 2 * tp + 2, 0:128],
                            kv_sb[
                                :, 2 * tp : 2 * tp + 2, (2 * k) * 512 : (2 * k + 1) * 512
                            ],
                            start=st,
                            stop=sp,
                            perf_mode=PM.DoubleRow,
                        )
                        mB = nc.tensor.matmul(
                            bB[0:128, 0:512],
                            fT[:, 2 * tp : 2 * tp + 2, 0:128],
                            kv_sb[
                                :,
                                2 * tp : 2 * tp + 2,
                                (2 * k + 1) * 512 : (2 * k + 2) * 512,
                            ],
                            start=st,
                            stop=sp,
                            perf_mode=PM.DoubleRow,
                        )
                    mA.then_inc(s_p, 1)
                    mB.then_inc(s_p, 1)

                p2triple(1)                       # s_p = 21, 22, 23
                p2pair(0, (2, 3), 0, 14, [0])     # s_p = 24, 25
                p2triple(2)                       # s_p = 26, 27, 28
                p2pair(1, (0, 1), 0, 14, [0, 1])  # s_p = 29, 30
                p2pair(2, (2, 3), 10, 17, [1])    # s_p = 31, 32
                p2pair(3, (0, 1), 13, 19, [1, 2])  # s_p = 33, 34
                p2triple(3)                       # s_p = 35
                p2pair(4, (2, 3), 14, 20, [2])    # s_p = 36, 37

            @block.scalar
            def _(act):
                # warm the exp table set early (reads the memset tile)
                act.wait_ge(s_g, 1)
                nc.scalar.activation(
                    scr[:, :], wz[:, 0:8], AF.Exp, bias=0.0, scale=1.0
                )
                act.wait_ge(s_i, 16)
                # e64 = 64 * exp(s * SCALE); bias AP holds ln(64)
                for g in range(3):
                    act.wait_ge(s_p, 1 + g)
                    nc.scalar.activation(
                        e_lo[32 * g : 32 * g + 32, 0:256],
                        M1B[g][32 * g : 32 * g + 32, 0:256],
                        AF.Exp,
                        bias=idc[32 * g : 32 * g + 32, 128:129],
                        scale=SCALE,
                    ).then_inc(s_a, 1)  # 1, 2, 3
                act.wait_ge(s_p, 4)
                nc.scalar.activation(
                    e_hi[:, 512:768], ps_b[0:128, 0:256], AF.Exp,
                    bias=idc[:, 128:129], scale=SCALE,
                ).then_inc(s_a, 1)  # 4
                act.wait_ge(s_p, 11)
                nc.scalar.activation(
                    e_hi[:, 0:512], ps_a[0:128, 0:512], AF.Exp,
                    bias=idc[:, 128:129], scale=SCALE,
                ).then_inc(s_a, 1)  # 5

                def m1drain(gi, r, spv):
                    act.wait_ge(s_p, spv)
                    nc.scalar.activation(
                        o_lo[32 * gi : 32 * gi + 32, r, :],
                        RB[gi][32 * gi : 32 * gi + 32, 0:512],
                        AF.Copy,
                    ).then_inc(s_a, 1)

                def ntdrain_a(g, spv, bank):
                    act.wait_ge(s_p, spv)
                    nc.scalar.activation(
                        o_hi[:, g * 512 : (g + 1) * 512],
                        bank[0:128, 0:512],
                        AF.Copy,
                    ).then_inc(s_a, 1)

                m1drain(0, 0, 12)            # 6
                m1drain(1, 0, 13)            # 7
                m1drain(0, 1, 21)            # 8
                m1drain(1, 1, 22)            # 9
                ntdrain_a(1, 25, ps_t[3])    # 10
                m1drain(0, 2, 26)            # 11
                m1drain(1, 2, 27)            # 12
                ntdrain_a(3, 30, ps_t[1])    # 13
                ntdrain_a(5, 32, ps_t[3])    # 14
                ntdrain_a(7, 34, ps_t[1])    # 15
                m1drain(0, 3, 35)            # 16
                ntdrain_a(9, 37, ps_t[3])    # 17

            @block.vector
            def _(dve):
                nc.vector.memset(wz[:, :], 0.0).then_inc(s_g, 1)
                dve.wait_ge(s_a, 3)
                nc.vector.reduce_sum(
                    dl_sb[:], e_lo[:, :], axis=mybir.AxisListType.X
                ).then_inc(s_v, 1)  # 1
                # fT copies: f64 = e64T - 64, cast to fp8
                for k in range(6):
                    dve.wait_ge(s_p, 5 + k)
                    nc.vector.tensor_scalar_sub(
                        fT[:, k, 128:160], TR1_BANK[k][0:128, 0:32], 64.0
                    ).then_inc(s_v, 1)  # 2..7
                def gc_drain(r, sp_val):
                    dve.wait_ge(s_p, sp_val)
                    nc.vector.tensor_copy(
                        o_lo[64:96, r, :], ps_m[64:96, 0:512]
                    ).then_inc(s_v, 1)

                def nt_drain(g, sp_val, bank):
                    dve.wait_ge(s_p, sp_val)
                    nc.vector.tensor_copy(
                        o_hi[:, g * 512 : (g + 1) * 512], bank[0:128, 0:512]
                    ).then_inc(s_v, 1)

                for k in range(2):
                    dve.wait_ge(s_p, 15 + k)
                    nc.vector.tensor_scalar_sub(
                        fT[:, TR0_CHUNK[k], 0:128], TR0_BANK[k][0:128, 0:128], 64.0
                    ).then_inc(s_v, 1)  # 8, 9
                gc_drain(0, 14)            # 10
                for k in range(2, 6):
                    dve.wait_ge(s_p, 15 + k)
                    nc.vector.tensor_scalar_sub(
                        fT[:, TR0_CHUNK[k], 0:128], TR0_BANK[k][0:128, 0:128], 64.0
                    ).then_inc(s_v, 1)  # 11..14
                dve.wait_ge(s_a, 5)
                nc.vector.reduce_sum(
                    dh_sb[:], e_hi[:, :], axis=mybir.AxisListType.X
                ).then_inc(s_v, 1)  # 15
                gc_drain(1, 23)            # 16
                nt_drain(0, 24, ps_t[2])   # 17
                gc_drain(2, 28)            # 18
                nt_drain(2, 29, ps_t[0])   # 19
                nt_drain(4, 31, ps_t[2])   # 20
                nt_drain(6, 33, ps_t[0])   # 21
                nt_drain(8, 36, ps_t[2])   # 22

    return nc


def _host_prep(z1_hat, z2):
    z1 = np.asarray(z1_hat, dtype=np.float32)[0]  # [128, 100, 64]
    z2a = np.asarray(z2, dtype=np.float32)[0]

    # q patches [160, 5120]; device layout q8 [128, j, i, 160]
    q = z1.reshape(KC, NH, KH, NW, KW).transpose(1, 3, 0, 2, 4).reshape(PQ, D)
    q4 = q.reshape(PQ, KC, KH, KW)
    q8 = q4.transpose(1, 3, 2, 0).astype(F8)  # [c, j, i, p]
    qm0 = np.ascontiguousarray(q8[:, :, :, 0:128])
    qm1 = np.ascontiguousarray(q8[:, :, :, 128:160])

    # padded z2: rows 100..111 zero
    z_pad = np.zeros((KC, 112, W), dtype=np.float32)
    z_pad[:, :H] = z2a
    z8_pad = z_pad.astype(F8)

    # sliding kv patches from padded z2 (original fp32 values, cast per-row)
    sw = np.lib.stride_tricks.sliding_window_view(z_pad, (KH, KW), axis=(1, 2))

    q64 = q.astype(np.float64)
    ij_off = (np.arange(KH)[:, None] * W + np.arange(KW)[None, :]).reshape(-1)  # [40]

    idc = np.zeros((128, 129), dtype=np.float32)
    idc[:, 0:128] = np.eye(128, dtype=np.float32)
    idc[:, 128] = LN64

    in_maps = []
    corrs = []
    for core in range(NCORES):
        h0 = HPC * core
        slab8 = z8_pad[:, h0 : h0 + ZROWS, :].reshape(KC, ZLEN)
        zz = np.zeros((KC, 2, ZLEN), dtype=F8)
        zz[:, 0, :] = slab8
        zz[:, 1, 0 : ZLEN - W] = slab8[:, W:]
        # kv rows indexed by flat position p = h_local*64 + w
        kvp = np.zeros((PKC, D), dtype=F8)
        hh = np.arange(PKC) // W
        ww = np.arange(PKC) % W
        real = (ww < WK) & (h0 + hh < HK)
        ridx = np.nonzero(real)[0]
        kvp[ridx] = (
            sw[:, h0 + hh[ridx], ww[ridx]].transpose(1, 0, 2, 3).reshape(-1, D)
        ).astype(F8)
        kv8 = np.ascontiguousarray(kvp.reshape(6, 128, D).transpose(1, 0, 2))
        in_maps.append(
            {"zz": zz, "qm0": qm0, "qm1": qm1, "kv8": kv8, "idc": idc}
        )
        # denominator correction: computed-but-invalid stream positions,
        # recomputed exactly (fp64) from the original values.
        bad = np.nonzero(~real)[0]
        zf = z_pad[:, h0 : h0 + ZROWS, :].reshape(KC, ZLEN).astype(np.float64)
        win = zf[:, bad[:, None] + ij_off[None, :]]  # [128, nb, 40]
        patches = win.transpose(1, 0, 2).reshape(len(bad), D)
        s_bad = q64 @ patches.T  # [160, nb]
        corrs.append(np.exp(s_bad * SCALE).sum(axis=1))

    corr = np.sum(corrs, axis=0)
    swr = sw[:, :HK, :WK]
    colsum = swr.astype(np.float64).sum(axis=(1, 2)).reshape(D)  # [5120]
    return in_maps, corr, colsum


def kernel(z1_hat, z2):
    from concourse.bass_utils import run_bass_kernel_spmd

    in_maps, corr, colsum = _host_prep(z1_hat, z2)
    if "nc" not in _CACHE:
        _CACHE["nc"] = _build_nc()
    nc = _CACHE["nc"]
    res = run_bass_kernel_spmd(nc, in_maps, list(range(NCORES)))
    num = np.broadcast_to(colsum, (PQ, D)).astype(np.float64).copy()
    den = -corr
    for r in res.results:
        ohi = r["ohi"].astype(np.float64)  # [128, 5120] = 64 * partial m0
        olo = r["olo"].astype(np.float64).reshape(96, 4, 512)
        part = np.zeros((PQ, D))
        part[0:128] = ohi
        for nt in range(10):
            rr, g = nt // 3, nt % 3
            part[128:160, nt * 512 : (nt + 1) * 512] = olo[32 * g : 32 * g + 32, rr]
        num += part / 64.0
        dv = r["den"].astype(np.float64)[:, 0] / 64.0
        den = den + np.concatenate(
            [dv[0:128], dv[128:160] + dv[160:192] + dv[192:224]]
        )
    out = (num / den[:, None]).astype(np.float32)
    # fold patches back: [160, 5120] -> [1, 128, 100, 64]
    out = out.reshape(NH, NW, KC, KH, KW).transpose(2, 0, 3, 1, 4)
    return np.ascontiguousarray(out.reshape(1, KC, H, W))
